# revision 1
# baseline (speedup 1.0000x reference)
"""Trainium2 Bass kernel for nn_NodeClassifier (gnn_message_passing).

Strategy (8 NeuronCores, SPMD):
  - Nodes block-partitioned by id across 8 cores (6250 each, padded to 6272).
    Within each core's block, nodes are sorted by in-degree so that the
    padded neighbor grid (K-grid) is tight.
  - Edges partitioned by dst core. Per dst node, its neighbor src tokens are
    laid out in a [128 nodes x K_t] index grid per 128-node tile; the src
    embeddings are fetched with indirect DMA gathers from a replicated
    node-embedding table in DRAM, then tree-reduced on the vector engine.
  - All per-node dense compute (GCN linear, BN, FF) runs feature-major
    ([128 features x nodes]) on the core's own shard. BN statistics are
    AllReduced (tiny). Between layers, the post-BN embeddings are AllGathered
    so every core has the full table for the next layer's gathers.
  - Weights replicated.

The program is identical on all cores (shared schedules = max over cores);
per-core behavior comes only from per-core input arrays.
"""

import os
import sys
import numpy as np

for _p in ("/opt/trn_rl_repo",):
    if _p not in sys.path and os.path.isdir(_p):
        sys.path.insert(0, _p)

from contextlib import ExitStack

import concourse.bass as bass
import concourse.bacc as bacc
import concourse.mybir as mybir
import concourse.tile as tile
from concourse.bass import IndirectOffsetOnAxis
from concourse.bass_utils import run_bass_kernel_spmd
from concourse.masks import make_identity

F32 = mybir.dt.float32
F16 = mybir.dt.float16
F32R = mybir.dt.float32r
I32 = mybir.dt.int32
AF = mybir.ActivationFunctionType
ALU = mybir.AluOpType

CORES = 8
D = 128
H = 512
DEPTH = 2
EPS = 1e-5
CHUNK = 512  # node-chunk width for the dense phase (one PSUM bank fp32)


# ----------------------------------------------------------------------------
# Host-side preparation
# ----------------------------------------------------------------------------

def _prepare(nodes, edge_src, edge_dst):
    """Compute the permutation, sharding and gather schedules from edge data."""
    N = nodes.shape[0]
    assert N % CORES == 0
    sh_real = N // CORES
    nt = -(-sh_real // 128)
    sh = nt * 128
    if sh == sh_real:  # force at least one dummy slot (PAD token row must be 0)
        nt += 1
        sh += 128
    tok_n = CORES * sh

    deg = np.bincount(edge_dst, minlength=N).astype(np.int64)

    # permutation: per core block, sort nodes by degree ascending
    tok_of_node = np.empty(N, np.int64)
    node_of_tok = np.full(tok_n, -1, np.int64)
    for c in range(CORES):
        ids = np.arange(c * sh_real, (c + 1) * sh_real)
        order = np.argsort(deg[ids], kind="stable")
        toks = c * sh + np.arange(sh_real)
        tok_of_node[ids[order]] = toks
        node_of_tok[toks] = ids[order]

    pad_tok = sh_real  # core 0's first dummy slot; its table row is zero

    # group edges by dst token
    dst_tok = tok_of_node[edge_dst]
    src_tok = tok_of_node[edge_src]
    order = np.argsort(dst_tok, kind="stable")
    dst_tok_s = dst_tok[order]
    src_tok_s = src_tok[order]
    cnt_tok = np.bincount(dst_tok_s, minlength=tok_n)
    start_tok = np.concatenate([[0], np.cumsum(cnt_tok)[:-1]])

    # shared K schedule: per tile index t, max over cores of max degree, even
    K_t = np.zeros(nt, np.int64)
    cnt_mat = cnt_tok.reshape(CORES, nt, 128)
    K_t = cnt_mat.max(axis=(0, 2))
    K_t = np.maximum(K_t, 2)
    K_t = K_t + (K_t % 2)
    koff = np.concatenate([[0], np.cumsum(K_t)])
    ksum = int(koff[-1])

    # per-core gather index grids [128, ksum] int32 (partition = node slot%128)
    gidx = np.full((CORES, 128, ksum), pad_tok, np.int32)
    t_of_slot = np.arange(sh) // 128
    e_slot = dst_tok_s % sh  # slot within core
    e_core = dst_tok_s // sh
    e_t = e_slot // 128
    e_p = e_slot % 128
    e_r = np.arange(len(dst_tok_s)) - start_tok[dst_tok_s]  # rank within node
    e_col = koff[e_t] + e_r
    gidx[e_core, e_p, e_col] = src_tok_s

    # per-core invdeg [128, nt] (0 for dummy slots)
    invdeg = np.zeros((CORES, 128, nt), np.float32)
    deg_tok = cnt_tok.reshape(CORES, sh)
    node_ok = (node_of_tok.reshape(CORES, sh) >= 0)
    iv = 1.0 / np.maximum(deg_tok, 1.0)
    iv = iv * node_ok
    for c in range(CORES):
        invdeg[c] = iv[c].reshape(nt, 128).T

    # replicated full node table [tok_n, D], zero at dummy slots
    table0 = np.zeros((tok_n, D), np.float32)
    real = node_of_tok >= 0
    table0[real] = nodes[node_of_tok[real]]

    # host-expanded layer-1 gather payload, fp16 [CORES][128, ksum*D]
    t16 = table0.astype(np.float16)
    pay1 = t16[gidx]  # [CORES, 128, ksum, D]
    pay1 = np.ascontiguousarray(pay1.reshape(CORES, 128, ksum * D))

    return dict(
        N=N, sh_real=sh_real, sh=sh, nt=nt, tok_n=tok_n,
        K_t=[int(k) for k in K_t], koff=[int(k) for k in koff], ksum=ksum,
        gidx=gidx, invdeg=invdeg, table0=table0, pay1=pay1,
        node_of_tok=node_of_tok,
    )


# ----------------------------------------------------------------------------
# Program builder
# ----------------------------------------------------------------------------

def _emit_tree_reduce(nc, G16, G32, K, acc):
    """acc = sum of K [128,D] fp16 chunks of G16. Pass 1 pairs fp16 halves
    into fp32 G32, then in-place fp32 halving on G32."""
    half = K // 2  # K is even
    if half == 1:
        nc.vector.tensor_tensor(out=acc[:], in0=G16[:, :D],
                                in1=G16[:, D:2 * D], op=ALU.add)
        return
    nc.vector.tensor_tensor(out=G32[:, :half * D], in0=G16[:, :half * D],
                            in1=G16[:, half * D:K * D], op=ALU.add)
    width = half
    while width > 2:
        h = width // 2
        if width % 2:
            nc.vector.tensor_tensor(
                out=G32[:, 0:D], in0=G32[:, 0:D],
                in1=G32[:, (width - 1) * D:width * D], op=ALU.add)
        if h == 1:  # width was 3: after the fold only chunks 0,1 remain
            break
        nc.vector.tensor_tensor(
            out=G32[:, :h * D], in0=G32[:, :h * D],
            in1=G32[:, h * D:2 * h * D], op=ALU.add)
        width = h
    nc.vector.tensor_tensor(out=acc[:], in0=G32[:, 0:D], in1=G32[:, D:2 * D],
                            op=ALU.add)


def build_program(cfg, debug=False):
    nt, sh, sh_real = cfg["nt"], cfg["sh"], cfg["sh_real"]
    tok_n, ksum = cfg["tok_n"], cfg["ksum"]
    K_t, koff = cfg["K_t"], cfg["koff"]
    N = cfg["N"]
    kmax = max(K_t)
    rg = [list(range(CORES))]

    chunks = []
    c0 = 0
    while c0 < sh:
        cw = min(CHUNK, sh - c0)
        chunks.append((c0, cw))
        c0 += cw
    nch = len(chunks)

    nc = bacc.Bacc("TRN2", target_bir_lowering=False, debug=False,
                   num_devices=CORES)

    # ---- I/O declarations
    pay1_d = nc.dram_tensor("pay1", [128, ksum * D], F16, kind="ExternalInput")
    x0_d = nc.dram_tensor("x0_fm", [D, sh], F32, kind="ExternalInput")
    gidx_d = nc.dram_tensor("gidx", [128, ksum], I32, kind="ExternalInput")
    invdeg_d = nc.dram_tensor("invdeg", [128, nt], F32, kind="ExternalInput")
    wg_d = [nc.dram_tensor(f"wg{l}", [D, D], F32, kind="ExternalInput")
            for l in range(DEPTH)]
    bgT_d = [nc.dram_tensor(f"bgT{l}", [1, D], F32, kind="ExternalInput")
             for l in range(DEPTH)]
    w1_d = [nc.dram_tensor(f"w1_{l}", [D, H], F32, kind="ExternalInput")
            for l in range(DEPTH)]
    fb1_d = [nc.dram_tensor(f"fb1_{l}", [D, H // D], F32, kind="ExternalInput")
             for l in range(DEPTH)]
    w2_d = [nc.dram_tensor(f"w2_{l}", [H, D], F32, kind="ExternalInput")
            for l in range(DEPTH)]
    bn_d = {}
    for l in range(DEPTH):
        for nm in ("g1", "b1", "g2", "b2"):
            bn_d[(nm, l)] = nc.dram_tensor(f"{nm}_{l}", [D, 1], F32,
                                           kind="ExternalInput")
    clsw_d = nc.dram_tensor("clsw", [D, 16], F32, kind="ExternalInput")
    clsb_d = nc.dram_tensor("clsb", [16, 1], F32, kind="ExternalInput")
    out_d = nc.dram_tensor("out_fm", [16, sh], F32, kind="ExternalOutput")
    dbg = {}
    if debug:
        for nm, shape, dt_ in [("dbg_agg0", [D, sh], F32),
                               ("dbg_u0", [D, sh], F32),
                               ("dbg_s2", [D, 2], F32),
                               ("dbg_sums", [D, 2], F32),
                               ("dbg_a1c1", [D, 2], F32),
                               ("dbg_v0", [D, sh], F32),
                               ("dbg_xnew0", [D, sh], F32),
                               ("dbg_vtab", [cfg["tok_n"], D], F16),
                               ("dbg_agg1", [D, sh], F32)]:
            dbg[nm] = nc.dram_tensor(nm, shape, dt_, kind="ExternalOutput")

    with tile.TileContext(nc) as tc, ExitStack() as ctx:
        dram = ctx.enter_context(tc.tile_pool(name="dram", bufs=1, space="DRAM"))
        wp = ctx.enter_context(tc.tile_pool(name="weights", bufs=1))
        big = ctx.enter_context(tc.tile_pool(name="big", bufs=1))
        gp = ctx.enter_context(tc.tile_pool(name="gather", bufs=2))
        sp = ctx.enter_context(tc.tile_pool(name="small", bufs=4))
        ck = ctx.enter_context(tc.tile_pool(name="chunk", bufs=2))
        psA = ctx.enter_context(tc.tile_pool(name="psA", bufs=1, space="PSUM"))
        psB = ctx.enter_context(tc.tile_pool(name="psB", bufs=2, space="PSUM"))

        # ---- internal DRAM (collective bounce buffers)
        vshard = dram.tile([sh, D], F16, name="vshard")
        vtab = dram.tile([tok_n, D], F16, addr_space="Shared", name="vtab")
        bn_in, bn_out = {}, {}
        for l in range(DEPTH):
            for j in (1, 2):
                bn_in[(l, j)] = dram.tile([D, 2], F32, name=f"bni{l}{j}")
                bn_out[(l, j)] = dram.tile([D, 2], F32, addr_space="Shared",
                                           name=f"bno{l}{j}")

        # ---- load constants / weights to SBUF
        def load(dt_, shape, src, name):
            t = wp.tile(shape, dt_, name=name)
            nc.sync.dma_start(out=t[:], in_=src)
            return t

        gidx_sb = load(I32, [128, ksum], gidx_d[:], "gidx_sb")
        invdeg_sb = load(F32, [128, nt], invdeg_d[:], "invdeg_sb")
        wg_sb = [load(F32, [D, D], wg_d[l][:], f"wg_sb{l}") for l in range(DEPTH)]
        bgT_sb = [load(F32, [1, D], bgT_d[l][:], f"bgT_sb{l}") for l in range(DEPTH)]
        w1_sb = [load(F32, [D, H], w1_d[l][:], f"w1_sb{l}") for l in range(DEPTH)]
        fb1_sb = [load(F32, [D, H // D], fb1_d[l][:], f"fb1_sb{l}")
                  for l in range(DEPTH)]
        w2_sb = [[load(F32, [D, D], w2_d[l][h * D:(h + 1) * D, :], f"w2_sb{l}_{h}")
                  for h in range(H // D)] for l in range(DEPTH)]
        bn_sb = {k: load(F32, [D, 1], v[:], f"bn_{k[0]}_{k[1]}")
                 for k, v in bn_d.items()}
        clsw_sb = load(F32, [D, 16], clsw_d[:], "clsw_sb")
        clsb_sb = load(F32, [16, 1], clsb_d[:], "clsb_sb")

        ident = wp.tile([128, 128], F32, name="ident")
        make_identity(nc, ident[:])
        ones_row = wp.tile([1, CHUNK], F32, name="ones_row")
        nc.vector.memset(ones_row[:], 1.0)

        # ---- persistent full-width activations (feature-major [D, sh])
        bufA = big.tile([D, sh], F32, name="bufA")  # agg / xp-src scratch
        bufB = big.tile([D, sh], F32, name="bufB")  # u(l0) -> xres2(l0)=xres(l1)
        bufC = big.tile([D, sh], F32, name="bufC")  # x0(l0) -> v(l1)
        bufD = big.tile([D, sh], F32, name="bufD")  # v(l0) -> u(l1)
        nc.sync.dma_start(out=bufC[:], in_=x0_d[:])

        def r32(ap):
            # float32r needs producer-side rounding (BIR verifier); plain
            # fp32 matmul for now. TODO: fp32r with rounded producers.
            return ap

        def bn_vec_math(sums_sb, g_sb, b_sb, a_out, c_out):
            """a = g*rsqrt(var+eps); c = b - mean*a, from [D,2] (sum, sumsq)."""
            m = sp.tile([D, 1], F32, tag="bnv", name="m")
            msq = sp.tile([D, 1], F32, tag="bnv", name="msq")
            var = sp.tile([D, 1], F32, tag="bnv", name="var")
            r = sp.tile([D, 1], F32, tag="bnv", name="r")
            nc.vector.tensor_scalar_mul(out=m[:], in0=sums_sb[:, 0:1],
                                        scalar1=1.0 / N)
            nc.vector.tensor_scalar_mul(out=msq[:], in0=sums_sb[:, 1:2],
                                        scalar1=1.0 / N)
            nc.vector.tensor_tensor(out=var[:], in0=m[:], in1=m[:], op=ALU.mult)
            nc.vector.tensor_tensor(out=var[:], in0=msq[:], in1=var[:],
                                    op=ALU.subtract)
            nc.vector.tensor_scalar_add(out=var[:], in0=var[:], scalar1=EPS)
            nc.vector.reciprocal(out=r[:], in_=var[:])
            nc.scalar.activation(out=a_out[:], in_=r[:], func=AF.Sqrt)
            nc.vector.tensor_tensor(out=a_out[:], in0=g_sb[:], in1=a_out[:],
                                    op=ALU.mult)
            nc.vector.tensor_tensor(out=c_out[:], in0=m[:], in1=a_out[:],
                                    op=ALU.mult)
            nc.vector.tensor_tensor(out=c_out[:], in0=b_sb[:], in1=c_out[:],
                                    op=ALU.subtract)

        def emit_stats_and_allreduce(src_buf, l, j, a_out, c_out):
            """Partial sum/sumsq of src_buf over real nodes -> AllReduce ->
            affine coefficients."""
            ssum = sp.tile([D, nch], F32, tag="stat", name=f"ssum{l}{j}")
            ssq = sp.tile([D, nch], F32, tag="stat", name=f"ssq{l}{j}")
            for ci, (c0, cw) in enumerate(chunks):
                rw = max(0, min(cw, sh_real - c0))
                if rw == 0:
                    nc.vector.memset(ssum[:, ci:ci + 1], 0.0)
                    nc.vector.memset(ssq[:, ci:ci + 1], 0.0)
                    continue
                sq = ck.tile([D, CHUNK], F32, tag="sq", name=f"sq{l}{j}{ci}")
                nc.vector.tensor_reduce(out=ssum[:, ci:ci + 1],
                                        in_=src_buf[:, c0:c0 + rw],
                                        axis=mybir.AxisListType.X, op=ALU.add)
                nc.vector.tensor_tensor(out=sq[:, :rw],
                                        in0=src_buf[:, c0:c0 + rw],
                                        in1=src_buf[:, c0:c0 + rw], op=ALU.mult)
                nc.vector.tensor_reduce(out=ssq[:, ci:ci + 1], in_=sq[:, :rw],
                                        axis=mybir.AxisListType.X, op=ALU.add)
            s2 = sp.tile([D, 2], F32, tag="s2", name=f"s2_{l}{j}")
            nc.vector.tensor_reduce(out=s2[:, 0:1], in_=ssum[:],
                                    axis=mybir.AxisListType.X, op=ALU.add)
            nc.vector.tensor_reduce(out=s2[:, 1:2], in_=ssq[:],
                                    axis=mybir.AxisListType.X, op=ALU.add)
            nc.sync.dma_start(out=bn_in[(l, j)][:], in_=s2[:])
            if dbg and l == 0 and j == 1:
                nc.sync.dma_start(out=dbg["dbg_s2"][:], in_=s2[:])
            nc.gpsimd.collective_compute(
                "AllReduce", ALU.add, replica_groups=rg,
                ins=[bn_in[(l, j)][:]], outs=[bn_out[(l, j)][:]])
            sums = sp.tile([D, 2], F32, tag="s2", name=f"sums{l}{j}")
            nc.sync.dma_start(out=sums[:], in_=bn_out[(l, j)][:])
            bn_vec_math(sums, bn_sb[(f"g{j}", l)], bn_sb[(f"b{j}", l)],
                        a_out, c_out)
            if dbg and l == 0 and j == 1:
                nc.sync.dma_start(out=dbg["dbg_sums"][:], in_=sums[:])
                nc.sync.dma_start(out=dbg["dbg_a1c1"][:, 0:1], in_=a_out[:])
                nc.sync.dma_start(out=dbg["dbg_a1c1"][:, 1:2], in_=c_out[:])

        for l in range(DEPTH):
            agg = bufA
            if l == 0:
                u, xres, v = bufB, bufC, bufD
            else:
                u, xres, v = bufD, bufB, bufC
            table = vtab

            # ---- aggregation: gather + tree reduce + invdeg + transpose
            for t in range(nt):
                K = K_t[t]
                G16 = gp.tile([128, kmax * D], F16, tag="G16", name=f"G{l}_{t}")
                if l == 0:
                    # layer-1 gather was expanded host-side; stream it
                    nc.sync.dma_start(
                        out=G16[:, :K * D],
                        in_=pay1_d[:, koff[t] * D:(koff[t] + K) * D])
                else:
                    for k in range(K):
                        nc.gpsimd.indirect_dma_start(
                            out=G16[:, k * D:(k + 1) * D], out_offset=None,
                            in_=table[:],
                            in_offset=IndirectOffsetOnAxis(
                                ap=gidx_sb[:, koff[t] + k:koff[t] + k + 1],
                                axis=0))
                G32 = gp.tile([128, (kmax // 2) * D], F32, tag="G32",
                              name=f"G32_{l}_{t}")
                acc = sp.tile([128, D], F32, tag="acc", name=f"acc{l}_{t}")
                _emit_tree_reduce(nc, G16, G32, K, acc)
                acc2 = sp.tile([128, D], F32, tag="acc2", name=f"acc2{l}_{t}")
                nc.vector.tensor_scalar_mul(out=acc2[:], in0=acc[:],
                                            scalar1=invdeg_sb[:, t:t + 1])
                ps = psB.tile([128, 128], F32, tag="tr", name=f"tr{l}_{t}")
                nc.tensor.transpose(ps[:], acc2[:], ident[:])
                nc.scalar.activation(out=agg[:, t * 128:(t + 1) * 128],
                                     in_=ps[:], func=AF.Copy)
            if dbg:
                nc.sync.dma_start(out=dbg["dbg_agg0" if l == 0 else "dbg_agg1"][:],
                                  in_=agg[:])

            # ---- dense sweep 1: GCN linear + residual -> u
            for c0, cw in chunks:
                sl = slice(c0, c0 + cw)
                ph = psA.tile([D, CHUNK], F32, tag="mm_gcn", name=f"ph{l}{c0}")
                nc.tensor.matmul(ph[:, :cw], r32(wg_sb[l][:]), r32(agg[:, sl]),
                                 start=True, stop=False)
                nc.tensor.matmul(ph[:, :cw], r32(bgT_sb[l][:]),
                                 r32(ones_row[:, :cw]), start=False, stop=True)
                nc.vector.tensor_tensor(out=u[:, sl], in0=ph[:, :cw],
                                        in1=xres[:, sl], op=ALU.add)

            if dbg and l == 0:
                nc.sync.dma_start(out=dbg["dbg_u0"][:], in_=u[:])
            a1 = sp.tile([D, 1], F32, tag="co", name=f"a1_{l}")
            c1 = sp.tile([D, 1], F32, tag="co", name=f"c1_{l}")
            emit_stats_and_allreduce(u, l, 1, a1, c1)

            # ---- dense sweep 2: BN1 affine -> FF -> v
            for c0, cw in chunks:
                sl = slice(c0, c0 + cw)
                xp = ck.tile([D, CHUNK], F32, tag="xp", name=f"xp{l}{c0}")
                nc.vector.tensor_scalar(out=xp[:, :cw], in0=u[:, sl],
                                        scalar1=a1[:], scalar2=c1[:],
                                        op0=ALU.mult, op1=ALU.add)
                py = psA.tile([D, CHUNK], F32, tag="mm_ff2", name=f"py{l}{c0}")
                for h in range(H // D):
                    pr = psA.tile([D, CHUNK], F32, tag=f"mm_ff1_{h}",
                                  name=f"pr{l}{c0}{h}")
                    nc.tensor.matmul(pr[:, :cw], r32(w1_sb[l][:, h * D:(h + 1) * D]),
                                     r32(xp[:, :cw]), start=True, stop=True)
                    rh = ck.tile([D, CHUNK], F32, tag=f"relu{h}",
                                 name=f"rh{l}{c0}{h}")
                    nc.scalar.activation(out=rh[:, :cw], in_=pr[:, :cw],
                                         func=AF.Relu, bias=fb1_sb[l][:, h:h + 1],
                                         scale=1.0)
                    nc.tensor.matmul(py[:, :cw], r32(w2_sb[l][h][:]),
                                     r32(rh[:, :cw]), start=(h == 0),
                                     stop=(h == H // D - 1))
                nc.vector.tensor_tensor(out=v[:, sl], in0=py[:, :cw],
                                        in1=xp[:, :cw], op=ALU.add)

            if dbg and l == 0:
                nc.sync.dma_start(out=dbg["dbg_v0"][:], in_=v[:])
            a2 = sp.tile([D, 1], F32, tag="co", name=f"a2_{l}")
            c2 = sp.tile([D, 1], F32, tag="co", name=f"c2_{l}")
            emit_stats_and_allreduce(v, l, 2, a2, c2)

            # ---- post-BN embeddings x'' (next residual / table / cls input)
            xnew = bufB if l == 0 else bufA
            for c0, cw in chunks:
                sl = slice(c0, c0 + cw)
                nc.vector.tensor_scalar(out=xnew[:, sl], in0=v[:, sl],
                                        scalar1=a2[:], scalar2=c2[:],
                                        op0=ALU.mult, op1=ALU.add)
            if sh > sh_real:
                nc.vector.memset(xnew[:, sh_real:sh], 0.0)

            if l == 0:
                # transpose to node-major, store shard, AllGather full table
                for t in range(nt):
                    ps = psB.tile([128, 128], F32, tag="tr", name=f"tv{t}")
                    nc.tensor.transpose(ps[:], xnew[:, t * 128:(t + 1) * 128],
                                        ident[:])
                    vT = sp.tile([128, D], F16, tag="vT", name=f"vT{t}")
                    nc.scalar.activation(out=vT[:], in_=ps[:], func=AF.Copy)
                    nc.sync.dma_start(out=vshard[t * 128:(t + 1) * 128, :],
                                      in_=vT[:])
                nc.gpsimd.collective_compute(
                    "AllGather", ALU.bypass, replica_groups=rg,
                    ins=[vshard[:]], outs=[vtab[:]])
                if dbg:
                    nc.sync.dma_start(out=dbg["dbg_xnew0"][:], in_=xnew[:])
                    nc.sync.dma_start(out=dbg["dbg_vtab"][:], in_=vtab[:])
            else:
                out_sb = wp.tile([16, sh], F32, name="out_sb")
                for c0, cw in chunks:
                    sl = slice(c0, c0 + cw)
                    pc = psA.tile([16, CHUNK], F32, tag="mm_gcn",
                                  name=f"pc{c0}")
                    nc.tensor.matmul(pc[:, :cw], r32(clsw_sb[:]),
                                     r32(xnew[:, sl]), start=True, stop=True)
                    nc.scalar.activation(out=out_sb[:, sl], in_=pc[:, :cw],
                                         func=AF.Identity, bias=clsb_sb[:],
                                         scale=1.0)
                nc.sync.dma_start(out=out_d[:], in_=out_sb[:])

    nc.compile()
    return nc


# ----------------------------------------------------------------------------
# Entry points
# ----------------------------------------------------------------------------

def _make_in_maps(cfg, inputs):
    W_gcn = np.asarray(inputs["W_gcn"], np.float32)
    b_gcn = np.asarray(inputs["b_gcn"], np.float32)
    ff_w1 = np.asarray(inputs["ff_w1"], np.float32)
    ff_b1 = np.asarray(inputs["ff_b1"], np.float32)
    ff_w2 = np.asarray(inputs["ff_w2"], np.float32)
    cls_w = np.asarray(inputs["cls_w"], np.float32)
    cls_b = np.asarray(inputs["cls_b"], np.float32)

    shared = {
        "clsw": np.ascontiguousarray(cls_w),
        "clsb": np.ascontiguousarray(cls_b.reshape(16, 1)),
    }
    for l in range(DEPTH):
        shared[f"wg{l}"] = np.ascontiguousarray(W_gcn[l])
        shared[f"bgT{l}"] = np.ascontiguousarray(b_gcn[l].reshape(1, D))
        shared[f"w1_{l}"] = np.ascontiguousarray(ff_w1[l])
        shared[f"fb1_{l}"] = np.ascontiguousarray(
            ff_b1[l].reshape(H // D, D).T)
        shared[f"w2_{l}"] = np.ascontiguousarray(ff_w2[l])
        shared[f"g1_{l}"] = np.ascontiguousarray(
            np.asarray(inputs["bn1_g"], np.float32)[l].reshape(D, 1))
        shared[f"b1_{l}"] = np.ascontiguousarray(
            np.asarray(inputs["bn1_b"], np.float32)[l].reshape(D, 1))
        shared[f"g2_{l}"] = np.ascontiguousarray(
            np.asarray(inputs["bn2_g"], np.float32)[l].reshape(D, 1))
        shared[f"b2_{l}"] = np.ascontiguousarray(
            np.asarray(inputs["bn2_b"], np.float32)[l].reshape(D, 1))

    sh = cfg["sh"]
    in_maps = []
    for c in range(CORES):
        m = dict(shared)
        m["x0_fm"] = np.ascontiguousarray(
            cfg["table0"][c * sh:(c + 1) * sh].T)
        m["pay1"] = cfg["pay1"][c]
        m["gidx"] = np.ascontiguousarray(cfg["gidx"][c])
        m["invdeg"] = np.ascontiguousarray(cfg["invdeg"][c])
        in_maps.append(m)
    return in_maps


def _postprocess(cfg, results):
    sh, sh_real = cfg["sh"], cfg["sh_real"]
    N = cfg["N"]
    node_of_tok = cfg["node_of_tok"]
    out = np.empty((N, 16), np.float32)
    for c in range(CORES):
        arr = results[c]["out_fm"]  # [16, sh]
        toks = np.arange(c * sh, c * sh + sh_real)
        out[node_of_tok[toks]] = arr.T[:sh_real]
    return out


def _ensure_axon_hooks():
    """The agent image's antenv lacks axon_hooks; synthesize it so
    bass_utils' trace=True path can find the NTFF profile hook."""
    try:
        import antenv.axon_hooks  # noqa: F401
        return
    except ImportError:
        pass
    import types
    import antenv
    mod = types.ModuleType("antenv.axon_hooks")
    mod._hook = None

    def set_axon_ntff_profile_hook(h):
        mod._hook = h

    def get_axon_ntff_profile_hook():
        return mod._hook

    mod.set_axon_ntff_profile_hook = set_axon_ntff_profile_hook
    mod.get_axon_ntff_profile_hook = get_axon_ntff_profile_hook
    sys.modules["antenv.axon_hooks"] = mod
    antenv.axon_hooks = mod
    try:
        from trn_agent_boot.trn_boot import _ntff_profile_via_ctypes
        h = _ntff_profile_via_ctypes("/opt/axon/libaxon_pjrt.so")
        if h is not None:
            mod._hook = h
    except Exception as e:  # pragma: no cover
        print(f"ntff hook setup failed: {e}", file=sys.stderr)


_CACHE = {}


def run(trace=False, **inputs):
    if trace:
        _ensure_axon_hooks()
    nodes = np.asarray(inputs["nodes"], np.float32)
    edge_src = np.asarray(inputs["edge_src"], np.int64)
    edge_dst = np.asarray(inputs["edge_dst"], np.int64)
    cfg = _prepare(nodes, edge_src, edge_dst)

    key = (nodes.shape, len(edge_src), tuple(cfg["K_t"]))
    if key not in _CACHE:
        _CACHE[key] = build_program(cfg)
    nc = _CACHE[key]

    in_maps = _make_in_maps(cfg, inputs)
    res = run_bass_kernel_spmd(nc, in_maps, list(range(CORES)), trace=trace)
    return _postprocess(cfg, res.results), res


def kernel(**inputs) -> np.ndarray:
    out, _ = run(trace=False, **inputs)
    return out



# revision 9
# speedup vs baseline: 1.3286x; 1.3286x over previous
"""Trainium2 Bass kernel for nn_NodeClassifier (gnn_message_passing).

Strategy (8 NeuronCores, SPMD):
  - Nodes block-partitioned by id across 8 cores (6250 each, padded to 6272).
  - Edges partitioned by dst core, grouped per 128-node dst tile, split into
    two src-token ranges (A: tok < 32768, B: rest) so gather indices fit
    int16, and padded to 128-edge blocks (block structure shared across
    cores = max over cores).
  - Aggregation is a segment-sum done ON THE TENSOR ENGINE: per 128-edge
    block, a [128 edges x 128 nodes] 0/1 selection matrix S (built on the
    vector engine from compact per-edge dst slots via iota==d) multiplies
    the gathered [128 edges x 128 feat] fp16 payload, accumulating into a
    per-tile PSUM [128 nodes x 128 feat]. A leading zero-matmul clears the
    accumulator so has_written semantics are never relied on.
  - Layer-0 payload is host-expanded (contiguous DMA streams). Layer-1
    payload is fetched with batched dma_gather (custom SWDGE instruction,
    1024 indices per call) from the AllGathered raw-v table.
  - All dense compute (GCN linear, BN, FF, cls) in fp16 matmuls,
    feature-major. BN stats via per-chunk DVE reduce + ACT Square accum_out,
    AllReduced (tiny). b_gcn dropped (BN(z+const)==BN(z), exact).
  - Layer-0 BN2 folded across the halo exchange: AllGather ships RAW v;
    a2 folds into layer-1's GCN weight, c2 via a rank-1 (c2^T W) x mask
    matmul and the local residual. Layer-1 BN2 folds into the classifier.
  - Weights replicated. Program identical on all cores.
"""

import os
import sys
import numpy as np

for _p in ("/opt/trn_rl_repo",):
    if _p not in sys.path and os.path.isdir(_p):
        sys.path.insert(0, _p)

from contextlib import ExitStack

import concourse.bass as bass
import concourse.bacc as bacc
import concourse.mybir as mybir
import concourse.tile as tile
from concourse.bass_utils import run_bass_kernel_spmd
from concourse.masks import make_identity

F32 = mybir.dt.float32
F16 = mybir.dt.float16
I16 = mybir.dt.int16
AF = mybir.ActivationFunctionType
ALU = mybir.AluOpType

CORES = 8
D = 128
H = 512
DEPTH = 2
EPS = 1e-5
CHUNK = 512
BOUND = 32768          # int16 gather-index range split
BLK = 128              # edges per gather/matmul block
GBLK = 8               # blocks per dma_gather (1024-descriptor ring cap)


# ----------------------------------------------------------------------------
# Host-side preparation
# ----------------------------------------------------------------------------

def _prepare(nodes, edge_src, edge_dst):
    N = nodes.shape[0]
    assert N % CORES == 0
    sh_real = N // CORES
    nt = -(-sh_real // 128)
    sh = nt * 128
    if sh == sh_real:
        nt += 1
        sh += 128
    tok_n = CORES * sh

    # permutation: per core block, sort nodes by degree ascending (keeps the
    # dense phase layout of the earlier kernel; not load-bearing here)
    deg = np.bincount(edge_dst, minlength=N).astype(np.int64)
    tok_of_node = np.empty(N, np.int64)
    node_of_tok = np.full(tok_n, -1, np.int64)
    for c in range(CORES):
        ids = np.arange(c * sh_real, (c + 1) * sh_real)
        order = np.argsort(deg[ids], kind="stable")
        toks = c * sh + np.arange(sh_real)
        tok_of_node[ids[order]] = toks
        node_of_tok[toks] = ids[order]

    dst_tok = tok_of_node[edge_dst]
    src_tok = tok_of_node[edge_src]

    e_core = dst_tok // sh
    e_slot = dst_tok % sh
    e_t = e_slot // 128
    e_p = e_slot % 128
    e_r = (src_tok >= BOUND).astype(np.int64)

    # per (core, tile, range) edge counts -> shared block counts
    cnt = np.zeros((CORES, nt, 2), np.int64)
    np.add.at(cnt, (e_core, e_t, e_r), 1)
    nblk_t = np.maximum(-(-cnt.max(axis=0) // BLK), 1)  # [nt, 2]
    blkoff = np.zeros((nt, 2), np.int64)
    nblkR = [0, 0]
    for r in range(2):
        off = 0
        for t in range(nt):
            blkoff[t, r] = off
            off += nblk_t[t, r]
        nblkR[r] = off

    # per-core edge placement: edge -> (range, global block, lane)
    # order within (core, tile, range): stable original order
    idx_arr = [np.zeros((CORES, nblkR[r] * BLK), np.int64) for r in range(2)]
    dloc = [np.full((CORES, nblkR[r] * BLK), 999.0, np.float32) for r in range(2)]
    paytok = [np.full((CORES, nblkR[r] * BLK), -1, np.int64) for r in range(2)]
    order = np.lexsort((np.arange(len(dst_tok)), e_r, e_t, e_core))
    # rank within (core, tile, range)
    key = ((e_core * nt + e_t) * 2 + e_r)
    ks = key[order]
    starts = np.searchsorted(ks, np.arange(CORES * nt * 2), side="left")
    rank = np.arange(len(order)) - starts[ks]
    ec, et, er, ep = e_core[order], e_t[order], e_r[order], e_p[order]
    st = src_tok[order]
    pos = (blkoff[et, er] * BLK + rank)
    for r in range(2):
        m = er == r
        idx_arr[r][ec[m], pos[m]] = st[m] - r * BOUND
        dloc[r][ec[m], pos[m]] = ep[m]
        paytok[r][ec[m], pos[m]] = st[m]

    # invdeg [128, nt] per core (0 for dummy slots)
    cnt_tok = np.bincount(dst_tok, minlength=tok_n)
    deg_tok = cnt_tok.reshape(CORES, sh)
    node_ok = node_of_tok.reshape(CORES, sh) >= 0
    iv = (1.0 / np.maximum(deg_tok, 1.0)) * node_ok
    mask = ((deg_tok > 0) & node_ok).astype(np.float16)
    invdeg = np.zeros((CORES, 128, nt), np.float32)
    for c in range(CORES):
        invdeg[c] = iv[c].reshape(nt, 128).T

    # replicated full node table [tok_n, D]
    table0 = np.zeros((tok_n, D), np.float32)
    real = node_of_tok >= 0
    table0[real] = nodes[node_of_tok[real]]
    t16 = table0.astype(np.float16)

    # layer-0 payload streams (edge-blocked x0 rows), fp16
    pay = []
    for r in range(2):
        p = np.zeros((CORES, nblkR[r] * BLK, D), np.float16)
        valid = paytok[r] >= 0
        p[valid] = t16[paytok[r][valid]]
        # gather layout: index i -> (partition i%128, block i//128)
        p = p.reshape(CORES, nblkR[r], BLK, D).transpose(0, 2, 1, 3)
        pay.append(np.ascontiguousarray(p.reshape(CORES, 128, nblkR[r] * D)))

    # wrapped int16 index arrays [128, nblkR*8] (replicated across 8 Q7 cores)
    idxw = []
    for r in range(2):
        w = idx_arr[r].reshape(CORES, -1, 16)  # [C, nblk*8, 16]
        w = w.transpose(0, 2, 1).astype(np.int16)  # [C, 16, nblk*8]
        idxw.append(np.ascontiguousarray(np.tile(w, (1, 8, 1))))

    # d_rel [128, nsub] fp16: per tile, its A-blocks then B-blocks
    # (lane -> partition)
    nsub_t = nblk_t.sum(axis=1)
    suboff = np.concatenate([[0], np.cumsum(nsub_t)])
    nsub = int(suboff[-1])
    drel = np.zeros((CORES, 128, nsub), np.float16)
    for t in range(nt):
        s0 = suboff[t]
        for r in range(2):
            b0, nb = blkoff[t, r], nblk_t[t, r]
            seg = dloc[r][:, b0 * BLK:(b0 + nb) * BLK].reshape(CORES, nb, BLK)
            drel[:, :, s0:s0 + nb] = seg.transpose(0, 2, 1).astype(np.float16)
            s0 += nb

    maxsub = int(nsub_t.max())
    iota_rep = np.tile(np.arange(128, dtype=np.float16), maxsub)[None, :]
    iota_rep = np.ascontiguousarray(np.broadcast_to(
        iota_rep, (128, maxsub * 128)))

    return dict(
        N=N, sh_real=sh_real, sh=sh, nt=nt, tok_n=tok_n,
        nblk_t=nblk_t, blkoff=blkoff, nblkR=nblkR,
        nsub_t=[int(x) for x in nsub_t], suboff=[int(x) for x in suboff],
        maxsub=maxsub, iota_rep=iota_rep,
        idxw=idxw, pay=pay, drel=drel,
        invdeg=invdeg, mask=mask, table0=table0, node_of_tok=node_of_tok,
    )


# ----------------------------------------------------------------------------
# Program builder
# ----------------------------------------------------------------------------

def build_program(cfg):
    nt, sh, sh_real = cfg["nt"], cfg["sh"], cfg["sh_real"]
    tok_n = cfg["tok_n"]
    nblk_t, blkoff, nblkR = cfg["nblk_t"], cfg["blkoff"], cfg["nblkR"]
    nsub_t, suboff, maxsub = cfg["nsub_t"], cfg["suboff"], cfg["maxsub"]
    N = cfg["N"]
    rg = [list(range(CORES))]

    chunks = []
    c0 = 0
    while c0 < sh:
        cw = min(CHUNK, sh - c0)
        chunks.append((c0, cw))
        c0 += cw
    nch = len(chunks)

    nc = bacc.Bacc("TRN2", target_bir_lowering=False, debug=False,
                   num_devices=CORES)

    # ---- I/O declarations
    pay_d = [nc.dram_tensor(f"pay{r}", [128, nblkR[r] * D], F16,
                            kind="ExternalInput") for r in range(2)]
    idx_d = [nc.dram_tensor(f"idx{r}", [128, nblkR[r] * 8], I16,
                            kind="ExternalInput") for r in range(2)]
    drel_d = nc.dram_tensor("drel", [128, suboff[-1]], F16,
                            kind="ExternalInput")
    iota_d = nc.dram_tensor("iotar", [128, maxsub * 128], F16,
                            kind="ExternalInput")
    x0_d = nc.dram_tensor("x016", [D, sh], F16, kind="ExternalInput")
    invdeg_d = nc.dram_tensor("invdeg", [128, nt], F32, kind="ExternalInput")
    mask_d = nc.dram_tensor("mask16", [1, sh], F16, kind="ExternalInput")
    wg_d = [nc.dram_tensor(f"wg{l}", [D, D], F16, kind="ExternalInput")
            for l in range(DEPTH)]
    w1_d = [nc.dram_tensor(f"w1_{l}", [D, H], F16, kind="ExternalInput")
            for l in range(DEPTH)]
    fb1_d = [nc.dram_tensor(f"fb1_{l}", [D, H // D], F32, kind="ExternalInput")
             for l in range(DEPTH)]
    w2_d = [nc.dram_tensor(f"w2_{l}", [H, D], F16, kind="ExternalInput")
            for l in range(DEPTH)]
    bn_d = {}
    for l in range(DEPTH):
        for nm in ("g1", "b1", "g2", "b2"):
            bn_d[(nm, l)] = nc.dram_tensor(f"{nm}_{l}", [D, 1], F32,
                                           kind="ExternalInput")
    clsw_d = nc.dram_tensor("clsw", [D, 16], F16, kind="ExternalInput")
    clsb_d = nc.dram_tensor("clsb", [16, 1], F32, kind="ExternalInput")
    out_d = nc.dram_tensor("out_fm", [16, sh], F32, kind="ExternalOutput")

    with tile.TileContext(nc) as tc, ExitStack() as ctx:
        dram = ctx.enter_context(tc.tile_pool(name="dram", bufs=1, space="DRAM"))
        wp = ctx.enter_context(tc.tile_pool(name="weights", bufs=1))
        big = ctx.enter_context(tc.tile_pool(name="big", bufs=1))
        gp = ctx.enter_context(tc.tile_pool(name="gather", bufs=8))
        sp = ctx.enter_context(tc.tile_pool(name="small", bufs=4))
        ck = ctx.enter_context(tc.tile_pool(name="chunk", bufs=2))
        psA = ctx.enter_context(tc.tile_pool(name="psA", bufs=2, space="PSUM"))
        psG = ctx.enter_context(tc.tile_pool(name="psG", bufs=2, space="PSUM"))
        psF = ctx.enter_context(tc.tile_pool(name="psF", bufs=2, space="PSUM"))
        psY = ctx.enter_context(tc.tile_pool(name="psY", bufs=1, space="PSUM"))
        psT = ctx.enter_context(tc.tile_pool(name="psT", bufs=1, space="PSUM"))

        vshard = dram.tile([sh, D], F16, name="vshard")
        vtab = dram.tile([tok_n, D], F16, addr_space="Shared", name="vtab")
        bn_in, bn_out = {}, {}
        for l in range(DEPTH):
            for j in (1, 2):
                bn_in[(l, j)] = dram.tile([D, 2], F32, name=f"bni{l}{j}")
                bn_out[(l, j)] = dram.tile([D, 2], F32, addr_space="Shared",
                                           name=f"bno{l}{j}")

        def load(dt_, shape, src, name):
            t = wp.tile(shape, dt_, name=name)
            nc.sync.dma_start(out=t[:], in_=src)
            return t

        idx_sb = [load(I16, [128, nblkR[r] * 8], idx_d[r][:], f"idx_sb{r}")
                  for r in range(2)]
        drel_sb = load(F16, [128, suboff[-1]], drel_d[:], "drel_sb")
        iota_sb = load(F16, [128, maxsub * 128], iota_d[:], "iota_sb")
        invdeg_sb = load(F32, [128, nt], invdeg_d[:], "invdeg_sb")
        mask_sb = load(F16, [1, sh], mask_d[:], "mask_sb")
        wg_sb = [load(F16, [D, D], wg_d[l][:], f"wg_sb{l}") for l in range(DEPTH)]
        w1_sb = [load(F16, [D, H], w1_d[l][:], f"w1_sb{l}") for l in range(DEPTH)]
        fb1_sb = [load(F32, [D, H // D], fb1_d[l][:], f"fb1_sb{l}")
                  for l in range(DEPTH)]
        w2_sb = [[load(F16, [D, D], w2_d[l][h * D:(h + 1) * D, :], f"w2_sb{l}_{h}")
                  for h in range(H // D)] for l in range(DEPTH)]
        bn_sb = {k: load(F32, [D, 1], v[:], f"bn_{k[0]}_{k[1]}")
                 for k, v in bn_d.items()}
        clsw_sb = load(F16, [D, 16], clsw_d[:], "clsw_sb")
        clsb_sb = load(F32, [16, 1], clsb_d[:], "clsb_sb")

        ident16 = wp.tile([128, 128], F16, name="ident16")
        make_identity(nc, ident16[:])
        zeros16 = wp.tile([128, 128], F16, name="zeros16")
        nc.vector.memset(zeros16[:], 0.0)

        wg1p = wp.tile([D, D], F16, name="wg1p")
        cw2_16 = wp.tile([1, D], F16, name="cw2_16")
        clsw2 = wp.tile([D, 16], F16, name="clsw2")
        clsb2 = wp.tile([16, 1], F32, name="clsb2")

        agg16 = big.tile([D, sh], F16, name="agg16")
        u16 = big.tile([D, sh], F16, name="u16")
        v16 = big.tile([D, sh], F16, name="v16")
        xr16 = big.tile([D, sh], F16, name="xr16")
        nc.sync.dma_start(out=xr16[:], in_=x0_d[:])

        def bn_coeffs(l, j, s2, a_out, c_out):
            nc.sync.dma_start(out=bn_in[(l, j)][:], in_=s2[:])
            nc.gpsimd.collective_compute(
                "AllReduce", ALU.add, replica_groups=rg,
                ins=[bn_in[(l, j)][:]], outs=[bn_out[(l, j)][:]])
            sums = sp.tile([D, 2], F32, tag="sums", name=f"sums{l}{j}")
            nc.sync.dma_start(out=sums[:], in_=bn_out[(l, j)][:])
            g_sb = bn_sb[(f"g{j}", l)]
            b_sb = bn_sb[(f"b{j}", l)]
            m = sp.tile([D, 1], F32, tag="bnv", name="m")
            msq = sp.tile([D, 1], F32, tag="bnv", name="msq")
            var = sp.tile([D, 1], F32, tag="bnv", name="var")
            r_ = sp.tile([D, 1], F32, tag="bnv", name="r")
            nc.vector.tensor_scalar_mul(out=m[:], in0=sums[:, 0:1],
                                        scalar1=1.0 / N)
            nc.vector.tensor_scalar_mul(out=msq[:], in0=sums[:, 1:2],
                                        scalar1=1.0 / N)
            nc.vector.tensor_tensor(out=var[:], in0=m[:], in1=m[:], op=ALU.mult)
            nc.vector.tensor_tensor(out=var[:], in0=msq[:], in1=var[:],
                                    op=ALU.subtract)
            nc.vector.tensor_scalar_add(out=var[:], in0=var[:], scalar1=EPS)
            nc.vector.reciprocal(out=r_[:], in_=var[:])
            nc.scalar.activation(out=a_out[:], in_=r_[:], func=AF.Sqrt)
            nc.vector.tensor_tensor(out=a_out[:], in0=g_sb[:], in1=a_out[:],
                                    op=ALU.mult)
            nc.vector.tensor_tensor(out=c_out[:], in0=m[:], in1=a_out[:],
                                    op=ALU.mult)
            nc.vector.tensor_tensor(out=c_out[:], in0=b_sb[:], in1=c_out[:],
                                    op=ALU.subtract)

        nchunks_r = [-(-nblkR[r] // GBLK) for r in range(2)]

        for l in range(DEPTH):
            # ---- payload: stream (l=0) or batched dma_gather (l=1)
            gtiles = [[], []]
            for r in range(2):
                for j in range(nchunks_r[r]):
                    nb = min(GBLK, nblkR[r] - GBLK * j)
                    gt = gp.tile([128, GBLK * D], F16, tag=f"G{r}",
                                 name=f"G{l}_{r}_{j}")
                    gtiles[r].append(gt)
                    if l == 0:
                        nc.sync.dma_start(
                            out=gt[:, :nb * D],
                            in_=pay_d[r][:, GBLK * j * D:(GBLK * j + nb) * D])
                    else:
                        view = vtab[0:BOUND, :] if r == 0 else vtab[BOUND:tok_n, :]
                        nidx = nb * BLK
                        nc.gpsimd.dma_gather(
                            gt[:, :nb * D].rearrange("p (b d) -> p b d", d=D),
                            view, idx_sb[r][:, GBLK * 8 * j:GBLK * 8 * j + nb * 8],
                            nidx, nidx, D)
                nc_dummy = None  # noqa

            # ---- per-tile segment-matmul aggregation
            for t in range(nt):
                nbt = nsub_t[t]
                s0 = suboff[t]
                St = ck.tile([128, maxsub * 128], F16, tag="S", name=f"S{l}_{t}")
                dr = drel_sb[:, s0:s0 + nbt]
                dr_b = bass.AP(dr.tensor, dr.offset, dr.ap + [[0, 128]])
                nc.vector.tensor_tensor(
                    out=St[:, :nbt * 128].rearrange("p (b j) -> p b j", j=128),
                    in0=iota_sb[:, :nbt * 128].rearrange("p (b j) -> p b j", j=128),
                    in1=dr_b, op=ALU.is_equal)
                ps = psA.tile([128, D], F32, tag="agg", name=f"agg{l}_{t}")
                nc.tensor.matmul(ps[:], zeros16[:], zeros16[:],
                                 start=True, stop=False)
                si = 0
                for r in range(2):
                    b0, nb = int(blkoff[t][r]), int(nblk_t[t][r])
                    for bi in range(nb):
                        gb = b0 + bi
                        gt = gtiles[r][gb // GBLK]
                        slot = gb % GBLK
                        nc.tensor.matmul(
                            ps[:], St[:, si * 128:(si + 1) * 128],
                            gt[:, slot * D:(slot + 1) * D],
                            start=False, stop=(si == nbt - 1))
                        si += 1
                acc2 = sp.tile([128, D], F16, tag="acc2", name=f"acc2{l}_{t}")
                nc.vector.tensor_scalar_mul(out=acc2[:], in0=ps[:],
                                            scalar1=invdeg_sb[:, t:t + 1])
                pv = psT.tile([128, 128], F16, tag="tr", name=f"tr{l}_{t}")
                nc.tensor.transpose(pv[:], acc2[:], ident16[:])
                nc.scalar.activation(out=agg16[:, t * 128:(t + 1) * 128],
                                     in_=pv[:], func=AF.Copy)

            # ---- dense sweep 1: GCN linear + residual -> u; stats of u
            ssum1 = sp.tile([D, nch], F32, tag="ssum", name=f"ssum{l}1")
            ssq1 = sp.tile([D, nch], F32, tag="ssq", name=f"ssq{l}1")
            for ci, (c0, cw) in enumerate(chunks):
                sl = slice(c0, c0 + cw)
                ph = psG.tile([D, CHUNK], F32, tag="gcn", name=f"ph{l}{c0}")
                if l == 0:
                    nc.tensor.matmul(ph[:, :cw], wg_sb[0][:], agg16[:, sl],
                                     start=True, stop=True)
                else:
                    nc.tensor.matmul(ph[:, :cw], wg1p[:], agg16[:, sl],
                                     start=True, stop=False)
                    nc.tensor.matmul(ph[:, :cw], cw2_16[:], mask_sb[:, sl],
                                     start=False, stop=True)
                nc.vector.tensor_tensor(out=u16[:, sl], in0=ph[:, :cw],
                                        in1=xr16[:, sl], op=ALU.add)
                rw = max(0, min(cw, sh_real - c0))
                if rw == 0:
                    nc.vector.memset(ssum1[:, ci:ci + 1], 0.0)
                    nc.vector.memset(ssq1[:, ci:ci + 1], 0.0)
                    continue
                nc.vector.tensor_reduce(out=ssum1[:, ci:ci + 1],
                                        in_=u16[:, c0:c0 + rw],
                                        axis=mybir.AxisListType.X, op=ALU.add)
                sq = ck.tile([D, CHUNK], F16, tag="sq", name=f"sq{l}1{ci}")
                nc.scalar.activation(out=sq[:, :rw], in_=u16[:, c0:c0 + rw],
                                     func=AF.Square,
                                     accum_out=ssq1[:, ci:ci + 1])
            s2a = sp.tile([D, 2], F32, tag="s2", name=f"s2a{l}")
            nc.vector.tensor_reduce(out=s2a[:, 0:1], in_=ssum1[:],
                                    axis=mybir.AxisListType.X, op=ALU.add)
            nc.vector.tensor_reduce(out=s2a[:, 1:2], in_=ssq1[:],
                                    axis=mybir.AxisListType.X, op=ALU.add)
            a1 = sp.tile([D, 1], F32, tag="co", name=f"a1_{l}")
            c1 = sp.tile([D, 1], F32, tag="co", name=f"c1_{l}")
            bn_coeffs(l, 1, s2a, a1, c1)

            # ---- dense sweep 2: BN1 affine -> FF -> v; stats; (l=0) vshard
            ssum2 = sp.tile([D, nch], F32, tag="ssum", name=f"ssum{l}2")
            ssq2 = sp.tile([D, nch], F32, tag="ssq", name=f"ssq{l}2")
            for ci, (c0, cw) in enumerate(chunks):
                sl = slice(c0, c0 + cw)
                xp = ck.tile([D, CHUNK], F16, tag="xp", name=f"xp{l}{c0}")
                nc.vector.tensor_scalar(out=xp[:, :cw], in0=u16[:, sl],
                                        scalar1=a1[:], scalar2=c1[:],
                                        op0=ALU.mult, op1=ALU.add)
                py = psY.tile([D, CHUNK], F32, tag="ff2", name=f"py{l}{c0}")
                for h in range(H // D):
                    pr = psF.tile([D, CHUNK], F32, tag="ff1",
                                  name=f"pr{l}{c0}{h}")
                    nc.tensor.matmul(pr[:, :cw], w1_sb[l][:, h * D:(h + 1) * D],
                                     xp[:, :cw], start=True, stop=True)
                    rh = ck.tile([D, CHUNK], F16, tag="rh", name=f"rh{l}{c0}{h}")
                    nc.scalar.activation(out=rh[:, :cw], in_=pr[:, :cw],
                                         func=AF.Relu, bias=fb1_sb[l][:, h:h + 1],
                                         scale=1.0)
                    nc.tensor.matmul(py[:, :cw], w2_sb[l][h][:], rh[:, :cw],
                                     start=(h == 0), stop=(h == H // D - 1))
                nc.vector.tensor_tensor(out=v16[:, sl], in0=py[:, :cw],
                                        in1=xp[:, :cw], op=ALU.add)
                rw = max(0, min(cw, sh_real - c0))
                if l == 0 and rw < cw:
                    nc.vector.memset(v16[:, c0 + rw:c0 + cw], 0.0)
                if rw > 0:
                    nc.vector.tensor_reduce(out=ssum2[:, ci:ci + 1],
                                            in_=v16[:, c0:c0 + rw],
                                            axis=mybir.AxisListType.X,
                                            op=ALU.add)
                    sq = ck.tile([D, CHUNK], F16, tag="sq", name=f"sq{l}2{ci}")
                    nc.scalar.activation(out=sq[:, :rw], in_=v16[:, c0:c0 + rw],
                                         func=AF.Square,
                                         accum_out=ssq2[:, ci:ci + 1])
                else:
                    nc.vector.memset(ssum2[:, ci:ci + 1], 0.0)
                    nc.vector.memset(ssq2[:, ci:ci + 1], 0.0)
                if l == 0:
                    for t in range(c0 // 128, (c0 + cw) // 128):
                        pv = psT.tile([128, 128], F16, tag="tr", name=f"tv{t}")
                        nc.tensor.transpose(pv[:], v16[:, t * 128:(t + 1) * 128],
                                            ident16[:])
                        vT = sp.tile([128, D], F16, tag="vT", name=f"vT{t}")
                        nc.scalar.activation(out=vT[:], in_=pv[:], func=AF.Copy)
                        nc.sync.dma_start(out=vshard[t * 128:(t + 1) * 128, :],
                                          in_=vT[:])
            s2b = sp.tile([D, 2], F32, tag="s2", name=f"s2b{l}")
            nc.vector.tensor_reduce(out=s2b[:, 0:1], in_=ssum2[:],
                                    axis=mybir.AxisListType.X, op=ALU.add)
            nc.vector.tensor_reduce(out=s2b[:, 1:2], in_=ssq2[:],
                                    axis=mybir.AxisListType.X, op=ALU.add)

            if l == 0:
                nc.gpsimd.collective_compute(
                    "AllGather", ALU.bypass, replica_groups=rg,
                    ins=[vshard[:]], outs=[vtab[:]])
                a2 = sp.tile([D, 1], F32, tag="co", name="a2_0")
                c2 = sp.tile([D, 1], F32, tag="co", name="c2_0")
                bn_coeffs(l, 2, s2b, a2, c2)
                nc.vector.tensor_scalar_mul(out=wg1p[:], in0=wg_sb[1][:],
                                            scalar1=a2[:])
                c2_16 = sp.tile([D, 1], F16, tag="c216", name="c2_16")
                nc.vector.tensor_copy(out=c2_16[:], in_=c2[:])
                pcw = psG.tile([D, CHUNK], F32, tag="gcn", name="pcw2")
                nc.tensor.matmul(pcw[0:1, 0:D], c2_16[:], wg_sb[1][:],
                                 start=True, stop=True)
                nc.scalar.activation(out=cw2_16[:], in_=pcw[0:1, 0:D],
                                     func=AF.Copy)
                nc.vector.tensor_scalar(out=xr16[:], in0=v16[:],
                                        scalar1=a2[:], scalar2=c2[:],
                                        op0=ALU.mult, op1=ALU.add)
            else:
                a2p = sp.tile([D, 1], F32, tag="co", name="a2_1")
                c2p = sp.tile([D, 1], F32, tag="co", name="c2_1")
                bn_coeffs(l, 2, s2b, a2p, c2p)
                nc.vector.tensor_scalar_mul(out=clsw2[:], in0=clsw_sb[:],
                                            scalar1=a2p[:])
                c2p_16 = sp.tile([D, 1], F16, tag="c216", name="c2p_16")
                nc.vector.tensor_copy(out=c2p_16[:], in_=c2p[:])
                pcb = psY.tile([D, CHUNK], F32, tag="ff2", name="pcb")
                nc.tensor.matmul(pcb[0:16, 0:1], clsw_sb[:], c2p_16[:],
                                 start=True, stop=True)
                nc.vector.tensor_tensor(out=clsb2[:], in0=pcb[0:16, 0:1],
                                        in1=clsb_sb[:], op=ALU.add)
                for c0, cw in chunks:
                    sl = slice(c0, c0 + cw)
                    pc = psY.tile([D, CHUNK], F32, tag="ff2", name=f"pc{c0}")
                    nc.tensor.matmul(pc[0:16, :cw], clsw2[:], v16[:, sl],
                                     start=True, stop=True)
                    oc = ck.tile([16, CHUNK], F32, tag="oc", name=f"oc{c0}")
                    nc.scalar.activation(out=oc[:, :cw], in_=pc[0:16, :cw],
                                         func=AF.Identity, bias=clsb2[:],
                                         scale=1.0)
                    nc.sync.dma_start(out=out_d[:, sl], in_=oc[:, :cw])

    nc.compile()
    return nc


# ----------------------------------------------------------------------------
# Entry points
# ----------------------------------------------------------------------------

def _make_in_maps(cfg, inputs):
    W_gcn = np.asarray(inputs["W_gcn"], np.float32)
    ff_w1 = np.asarray(inputs["ff_w1"], np.float32)
    ff_b1 = np.asarray(inputs["ff_b1"], np.float32)
    ff_w2 = np.asarray(inputs["ff_w2"], np.float32)
    cls_w = np.asarray(inputs["cls_w"], np.float32)
    cls_b = np.asarray(inputs["cls_b"], np.float32)

    shared = {
        "clsw": np.ascontiguousarray(cls_w.astype(np.float16)),
        "clsb": np.ascontiguousarray(cls_b.reshape(16, 1)),
        "iotar": cfg["iota_rep"],
    }
    for l in range(DEPTH):
        shared[f"wg{l}"] = np.ascontiguousarray(W_gcn[l].astype(np.float16))
        shared[f"w1_{l}"] = np.ascontiguousarray(ff_w1[l].astype(np.float16))
        shared[f"fb1_{l}"] = np.ascontiguousarray(
            ff_b1[l].reshape(H // D, D).T)
        shared[f"w2_{l}"] = np.ascontiguousarray(ff_w2[l].astype(np.float16))
        shared[f"g1_{l}"] = np.ascontiguousarray(
            np.asarray(inputs["bn1_g"], np.float32)[l].reshape(D, 1))
        shared[f"b1_{l}"] = np.ascontiguousarray(
            np.asarray(inputs["bn1_b"], np.float32)[l].reshape(D, 1))
        shared[f"g2_{l}"] = np.ascontiguousarray(
            np.asarray(inputs["bn2_g"], np.float32)[l].reshape(D, 1))
        shared[f"b2_{l}"] = np.ascontiguousarray(
            np.asarray(inputs["bn2_b"], np.float32)[l].reshape(D, 1))

    sh = cfg["sh"]
    in_maps = []
    for c in range(CORES):
        m = dict(shared)
        m["x016"] = np.ascontiguousarray(
            cfg["table0"][c * sh:(c + 1) * sh].T.astype(np.float16))
        m["pay0"] = cfg["pay"][0][c]
        m["pay1"] = cfg["pay"][1][c]
        m["idx0"] = cfg["idxw"][0][c]
        m["idx1"] = cfg["idxw"][1][c]
        m["drel"] = np.ascontiguousarray(cfg["drel"][c])
        m["invdeg"] = np.ascontiguousarray(cfg["invdeg"][c])
        m["mask16"] = np.ascontiguousarray(cfg["mask"][c].reshape(1, sh))
        in_maps.append(m)
    return in_maps


def _postprocess(cfg, results):
    sh, sh_real = cfg["sh"], cfg["sh_real"]
    N = cfg["N"]
    node_of_tok = cfg["node_of_tok"]
    out = np.empty((N, 16), np.float32)
    for c in range(CORES):
        arr = results[c]["out_fm"]
        toks = np.arange(c * sh, c * sh + sh_real)
        out[node_of_tok[toks]] = arr.T[:sh_real]
    return out


def _ensure_axon_hooks():
    try:
        import antenv.axon_hooks  # noqa: F401
        return
    except ImportError:
        pass
    import types
    import antenv
    mod = types.ModuleType("antenv.axon_hooks")
    mod._hook = None

    def set_axon_ntff_profile_hook(h):
        mod._hook = h

    def get_axon_ntff_profile_hook():
        return mod._hook

    mod.set_axon_ntff_profile_hook = set_axon_ntff_profile_hook
    mod.get_axon_ntff_profile_hook = get_axon_ntff_profile_hook
    sys.modules["antenv.axon_hooks"] = mod
    antenv.axon_hooks = mod
    try:
        from trn_agent_boot.trn_boot import _ntff_profile_via_ctypes
        h = _ntff_profile_via_ctypes("/opt/axon/libaxon_pjrt.so")
        if h is not None:
            mod._hook = h
    except Exception as e:  # pragma: no cover
        print(f"ntff hook setup failed: {e}", file=sys.stderr)


_CACHE = {}


def run(trace=False, **inputs):
    if trace:
        _ensure_axon_hooks()
    nodes = np.asarray(inputs["nodes"], np.float32)
    edge_src = np.asarray(inputs["edge_src"], np.int64)
    edge_dst = np.asarray(inputs["edge_dst"], np.int64)
    cfg = _prepare(nodes, edge_src, edge_dst)

    key = (nodes.shape, len(edge_src), int(cfg["suboff"][-1]))
    if key not in _CACHE:
        _CACHE[key] = build_program(cfg)
    nc = _CACHE[key]

    in_maps = _make_in_maps(cfg, inputs)
    res = run_bass_kernel_spmd(nc, in_maps, list(range(CORES)), trace=trace)
    return _postprocess(cfg, res.results), res


def kernel(**inputs) -> np.ndarray:
    out, _ = run(trace=False, **inputs)
    return out


# revision 11
# speedup vs baseline: 2.1600x; 1.6258x over previous
"""Trainium2 Bass kernel for nn_NodeClassifier (gnn_message_passing).

Strategy (8 NeuronCores, SPMD):
  - Nodes block-partitioned by id across 8 cores (6250 each, padded to 6272).
  - Edges partitioned by dst core, grouped per 128-node dst tile, split into
    two src-token ranges (A: tok < 32768, B: rest) so gather indices fit
    int16, and padded to 128-edge blocks (block structure shared across
    cores = max over cores).
  - Aggregation is a segment-sum done ON THE TENSOR ENGINE: per 128-edge
    block, a [128 edges x 128 nodes] 0/1 selection matrix S (built on the
    vector engine from compact per-edge dst slots via iota==d) multiplies
    the gathered [128 edges x 128 feat] fp16 payload, accumulating into a
    per-tile PSUM [128 nodes x 128 feat]. A leading zero-matmul clears the
    accumulator so has_written semantics are never relied on.
  - Layer-0 payload is host-expanded (contiguous DMA streams). Layer-1
    payload is fetched with batched dma_gather (custom SWDGE instruction,
    1024 indices per call) from the AllGathered raw-v table.
  - All dense compute (GCN linear, BN, FF, cls) in fp16 matmuls,
    feature-major. BN stats via per-chunk DVE reduce + ACT Square accum_out,
    AllReduced (tiny). b_gcn dropped (BN(z+const)==BN(z), exact).
  - Layer-0 BN2 folded across the halo exchange: AllGather ships RAW v;
    a2 folds into layer-1's GCN weight, c2 via a rank-1 (c2^T W) x mask
    matmul and the local residual. Layer-1 BN2 folds into the classifier.
  - Weights replicated. Program identical on all cores.
"""

import os
import sys
import numpy as np

for _p in ("/opt/trn_rl_repo",):
    if _p not in sys.path and os.path.isdir(_p):
        sys.path.insert(0, _p)

from contextlib import ExitStack

import concourse.bass as bass
import concourse.bacc as bacc
import concourse.mybir as mybir
import concourse.tile as tile
from concourse.bass_utils import run_bass_kernel_spmd
from concourse.masks import make_identity

F32 = mybir.dt.float32
F16 = mybir.dt.float16
I16 = mybir.dt.int16
AF = mybir.ActivationFunctionType
ALU = mybir.AluOpType

CORES = 8
D = 128
H = 512
DEPTH = 2
EPS = 1e-5
CHUNK = 512
BOUND = 32768          # int16 gather-index range split
BLK = 128              # edges per gather/matmul block
GBLK = 8               # blocks per dma_gather (1024-descriptor ring cap)


# ----------------------------------------------------------------------------
# Host-side preparation
# ----------------------------------------------------------------------------

def _prepare(nodes, edge_src, edge_dst):
    N = nodes.shape[0]
    assert N % CORES == 0
    sh_real = N // CORES
    nt = -(-sh_real // 128)
    sh = nt * 128
    if sh == sh_real:
        nt += 1
        sh += 128
    tok_n = CORES * sh

    # permutation: per core block, sort nodes by degree ascending (keeps the
    # dense phase layout of the earlier kernel; not load-bearing here)
    deg = np.bincount(edge_dst, minlength=N).astype(np.int64)
    tok_of_node = np.empty(N, np.int64)
    node_of_tok = np.full(tok_n, -1, np.int64)
    for c in range(CORES):
        ids = np.arange(c * sh_real, (c + 1) * sh_real)
        order = np.argsort(deg[ids], kind="stable")
        toks = c * sh + np.arange(sh_real)
        tok_of_node[ids[order]] = toks
        node_of_tok[toks] = ids[order]

    dst_tok = tok_of_node[edge_dst]
    src_tok = tok_of_node[edge_src]

    e_core = dst_tok // sh
    e_slot = dst_tok % sh
    e_t = e_slot // 128
    e_p = e_slot % 128
    e_r = (src_tok >= BOUND).astype(np.int64)

    # per (core, tile, range) edge counts -> shared block counts
    cnt = np.zeros((CORES, nt, 2), np.int64)
    np.add.at(cnt, (e_core, e_t, e_r), 1)
    nblk_t = np.maximum(-(-cnt.max(axis=0) // BLK), 1)  # [nt, 2]
    blkoff = np.zeros((nt, 2), np.int64)
    nblkR = [0, 0]
    for r in range(2):
        off = 0
        for t in range(nt):
            blkoff[t, r] = off
            off += nblk_t[t, r]
        nblkR[r] = off

    # per-core edge placement: edge -> (range, global block, lane)
    # order within (core, tile, range): stable original order
    idx_arr = [np.zeros((CORES, nblkR[r] * BLK), np.int64) for r in range(2)]
    dloc = [np.full((CORES, nblkR[r] * BLK), 999.0, np.float32) for r in range(2)]
    paytok = [np.full((CORES, nblkR[r] * BLK), -1, np.int64) for r in range(2)]
    order = np.lexsort((np.arange(len(dst_tok)), e_r, e_t, e_core))
    # rank within (core, tile, range)
    key = ((e_core * nt + e_t) * 2 + e_r)
    ks = key[order]
    starts = np.searchsorted(ks, np.arange(CORES * nt * 2), side="left")
    rank = np.arange(len(order)) - starts[ks]
    ec, et, er, ep = e_core[order], e_t[order], e_r[order], e_p[order]
    st = src_tok[order]
    pos = (blkoff[et, er] * BLK + rank)
    for r in range(2):
        m = er == r
        idx_arr[r][ec[m], pos[m]] = st[m] - r * BOUND
        dloc[r][ec[m], pos[m]] = ep[m]
        paytok[r][ec[m], pos[m]] = st[m]

    # invdeg [128, nt] per core (0 for dummy slots)
    cnt_tok = np.bincount(dst_tok, minlength=tok_n)
    deg_tok = cnt_tok.reshape(CORES, sh)
    node_ok = node_of_tok.reshape(CORES, sh) >= 0
    iv = (1.0 / np.maximum(deg_tok, 1.0)) * node_ok
    mask = ((deg_tok > 0) & node_ok).astype(np.float16)
    invdeg = np.zeros((CORES, 128, nt), np.float32)
    for c in range(CORES):
        invdeg[c] = iv[c].reshape(nt, 128).T

    # replicated full node table [tok_n, D]
    table0 = np.zeros((tok_n, D), np.float32)
    real = node_of_tok >= 0
    table0[real] = nodes[node_of_tok[real]]
    t16 = table0.astype(np.float16)

    # layer-0 payload streams (edge-blocked x0 rows), fp16
    pay = []
    for r in range(2):
        p = np.zeros((CORES, nblkR[r] * BLK, D), np.float16)
        valid = paytok[r] >= 0
        p[valid] = t16[paytok[r][valid]]
        # gather layout: index i -> (partition i%128, block i//128)
        p = p.reshape(CORES, nblkR[r], BLK, D).transpose(0, 2, 1, 3)
        pay.append(np.ascontiguousarray(p.reshape(CORES, 128, nblkR[r] * D)))

    # wrapped int16 index arrays [128, nblkR*8] (replicated across 8 Q7 cores)
    idxw = []
    for r in range(2):
        w = idx_arr[r].reshape(CORES, -1, 16)  # [C, nblk*8, 16]
        w = w.transpose(0, 2, 1).astype(np.int16)  # [C, 16, nblk*8]
        idxw.append(np.ascontiguousarray(np.tile(w, (1, 8, 1))))

    # d_rel [128, nsub] fp16: per tile, its A-blocks then B-blocks
    # (lane -> partition)
    nsub_t = nblk_t.sum(axis=1)
    suboff = np.concatenate([[0], np.cumsum(nsub_t)])
    nsub = int(suboff[-1])
    drel = np.zeros((CORES, 128, nsub), np.float16)
    for t in range(nt):
        s0 = suboff[t]
        for r in range(2):
            b0, nb = blkoff[t, r], nblk_t[t, r]
            seg = dloc[r][:, b0 * BLK:(b0 + nb) * BLK].reshape(CORES, nb, BLK)
            drel[:, :, s0:s0 + nb] = seg.transpose(0, 2, 1).astype(np.float16)
            s0 += nb

    maxsub = int(nsub_t.max())
    iota_rep = np.tile(np.arange(128, dtype=np.float16), maxsub)[None, :]
    iota_rep = np.ascontiguousarray(np.broadcast_to(
        iota_rep, (128, maxsub * 128)))

    return dict(
        N=N, sh_real=sh_real, sh=sh, nt=nt, tok_n=tok_n,
        nblk_t=nblk_t, blkoff=blkoff, nblkR=nblkR,
        nsub_t=[int(x) for x in nsub_t], suboff=[int(x) for x in suboff],
        maxsub=maxsub, iota_rep=iota_rep,
        idxw=idxw, pay=pay, drel=drel,
        invdeg=invdeg, mask=mask, table0=table0, node_of_tok=node_of_tok,
    )


# ----------------------------------------------------------------------------
# Program builder
# ----------------------------------------------------------------------------

def build_program(cfg):
    nt, sh, sh_real = cfg["nt"], cfg["sh"], cfg["sh_real"]
    tok_n = cfg["tok_n"]
    nblk_t, blkoff, nblkR = cfg["nblk_t"], cfg["blkoff"], cfg["nblkR"]
    nsub_t, suboff, maxsub = cfg["nsub_t"], cfg["suboff"], cfg["maxsub"]
    N = cfg["N"]
    rg = [list(range(CORES))]

    chunks = []
    c0 = 0
    while c0 < sh:
        cw = min(CHUNK, sh - c0)
        chunks.append((c0, cw))
        c0 += cw
    nch = len(chunks)

    nc = bacc.Bacc("TRN2", target_bir_lowering=False, debug=False,
                   num_devices=CORES, num_swdge_queues=4)

    # ---- I/O declarations
    pay_d = [nc.dram_tensor(f"pay{r}", [128, nblkR[r] * D], F16,
                            kind="ExternalInput") for r in range(2)]
    idx_d = [nc.dram_tensor(f"idx{r}", [128, nblkR[r] * 8], I16,
                            kind="ExternalInput") for r in range(2)]
    drel_d = nc.dram_tensor("drel", [128, suboff[-1]], F16,
                            kind="ExternalInput")
    iota_d = nc.dram_tensor("iotar", [128, maxsub * 128], F16,
                            kind="ExternalInput")
    x0_d = nc.dram_tensor("x016", [D, sh], F16, kind="ExternalInput")
    invdeg_d = nc.dram_tensor("invdeg", [128, nt], F32, kind="ExternalInput")
    mask_d = nc.dram_tensor("mask16", [1, sh], F16, kind="ExternalInput")
    wg_d = [nc.dram_tensor(f"wg{l}", [D, D], F16, kind="ExternalInput")
            for l in range(DEPTH)]
    w1_d = [nc.dram_tensor(f"w1_{l}", [D, H], F16, kind="ExternalInput")
            for l in range(DEPTH)]
    fb1_d = [nc.dram_tensor(f"fb1_{l}", [D, H // D], F32, kind="ExternalInput")
             for l in range(DEPTH)]
    w2_d = [nc.dram_tensor(f"w2_{l}", [H, D], F16, kind="ExternalInput")
            for l in range(DEPTH)]
    bn_d = {}
    for l in range(DEPTH):
        for nm in ("g1", "b1", "g2", "b2"):
            bn_d[(nm, l)] = nc.dram_tensor(f"{nm}_{l}", [D, 1], F32,
                                           kind="ExternalInput")
    clsw_d = nc.dram_tensor("clsw", [D, 16], F16, kind="ExternalInput")
    clsb_d = nc.dram_tensor("clsb", [16, 1], F32, kind="ExternalInput")
    out_d = nc.dram_tensor("out_fm", [16, sh], F32, kind="ExternalOutput")

    with tile.TileContext(nc) as tc, ExitStack() as ctx:
        dram = ctx.enter_context(tc.tile_pool(name="dram", bufs=1, space="DRAM"))
        wp = ctx.enter_context(tc.tile_pool(name="weights", bufs=1))
        big = ctx.enter_context(tc.tile_pool(name="big", bufs=1))
        gp = ctx.enter_context(tc.tile_pool(name="gather", bufs=8))
        sp = ctx.enter_context(tc.tile_pool(name="small", bufs=4))
        ck = ctx.enter_context(tc.tile_pool(name="chunk", bufs=2))
        psA = ctx.enter_context(tc.tile_pool(name="psA", bufs=2, space="PSUM"))
        psG = ctx.enter_context(tc.tile_pool(name="psG", bufs=2, space="PSUM"))
        psF = ctx.enter_context(tc.tile_pool(name="psF", bufs=2, space="PSUM"))
        psY = ctx.enter_context(tc.tile_pool(name="psY", bufs=1, space="PSUM"))
        psT = ctx.enter_context(tc.tile_pool(name="psT", bufs=1, space="PSUM"))

        vshard = dram.tile([sh, D], F16, name="vshard")
        vtab = dram.tile([tok_n, D], F16, addr_space="Shared", name="vtab")
        bn_in, bn_out = {}, {}
        for l in range(DEPTH):
            for j in (1, 2):
                bn_in[(l, j)] = dram.tile([D, 2], F32, name=f"bni{l}{j}")
                bn_out[(l, j)] = dram.tile([D, 2], F32, addr_space="Shared",
                                           name=f"bno{l}{j}")

        def load(dt_, shape, src, name):
            t = wp.tile(shape, dt_, name=name)
            nc.sync.dma_start(out=t[:], in_=src)
            return t

        idx_sb = [load(I16, [128, nblkR[r] * 8], idx_d[r][:], f"idx_sb{r}")
                  for r in range(2)]
        drel_sb = load(F16, [128, suboff[-1]], drel_d[:], "drel_sb")
        iota_sb = load(F16, [128, maxsub * 128], iota_d[:], "iota_sb")
        invdeg_sb = load(F32, [128, nt], invdeg_d[:], "invdeg_sb")
        mask_sb = load(F16, [1, sh], mask_d[:], "mask_sb")
        wg_sb = [load(F16, [D, D], wg_d[l][:], f"wg_sb{l}") for l in range(DEPTH)]
        w1_sb = [load(F16, [D, H], w1_d[l][:], f"w1_sb{l}") for l in range(DEPTH)]
        fb1_sb = [load(F32, [D, H // D], fb1_d[l][:], f"fb1_sb{l}")
                  for l in range(DEPTH)]
        w2_sb = [[load(F16, [D, D], w2_d[l][h * D:(h + 1) * D, :], f"w2_sb{l}_{h}")
                  for h in range(H // D)] for l in range(DEPTH)]
        bn_sb = {k: load(F32, [D, 1], v[:], f"bn_{k[0]}_{k[1]}")
                 for k, v in bn_d.items()}
        clsw_sb = load(F16, [D, 16], clsw_d[:], "clsw_sb")
        clsb_sb = load(F32, [16, 1], clsb_d[:], "clsb_sb")

        ident16 = wp.tile([128, 128], F16, name="ident16")
        make_identity(nc, ident16[:])
        zeros16 = wp.tile([128, 128], F16, name="zeros16")
        nc.vector.memset(zeros16[:], 0.0)

        wg1p = wp.tile([D, D], F16, name="wg1p")
        cw2_16 = wp.tile([1, D], F16, name="cw2_16")
        clsw2 = wp.tile([D, 16], F16, name="clsw2")
        clsb2 = wp.tile([16, 1], F32, name="clsb2")

        agg16 = big.tile([D, sh], F16, name="agg16")
        u16 = big.tile([D, sh], F16, name="u16")
        v16 = big.tile([D, sh], F16, name="v16")
        xr16 = big.tile([D, sh], F16, name="xr16")
        nc.sync.dma_start(out=xr16[:], in_=x0_d[:])

        def bn_coeffs(l, j, s2, a_out, c_out):
            nc.sync.dma_start(out=bn_in[(l, j)][:], in_=s2[:])
            nc.gpsimd.collective_compute(
                "AllReduce", ALU.add, replica_groups=rg,
                ins=[bn_in[(l, j)][:]], outs=[bn_out[(l, j)][:]])
            sums = sp.tile([D, 2], F32, tag="sums", name=f"sums{l}{j}")
            nc.sync.dma_start(out=sums[:], in_=bn_out[(l, j)][:])
            g_sb = bn_sb[(f"g{j}", l)]
            b_sb = bn_sb[(f"b{j}", l)]
            m = sp.tile([D, 1], F32, tag="bnv", name="m")
            msq = sp.tile([D, 1], F32, tag="bnv", name="msq")
            var = sp.tile([D, 1], F32, tag="bnv", name="var")
            r_ = sp.tile([D, 1], F32, tag="bnv", name="r")
            nc.vector.tensor_scalar_mul(out=m[:], in0=sums[:, 0:1],
                                        scalar1=1.0 / N)
            nc.vector.tensor_scalar_mul(out=msq[:], in0=sums[:, 1:2],
                                        scalar1=1.0 / N)
            nc.vector.tensor_tensor(out=var[:], in0=m[:], in1=m[:], op=ALU.mult)
            nc.vector.tensor_tensor(out=var[:], in0=msq[:], in1=var[:],
                                    op=ALU.subtract)
            nc.vector.tensor_scalar_add(out=var[:], in0=var[:], scalar1=EPS)
            nc.vector.reciprocal(out=r_[:], in_=var[:])
            nc.scalar.activation(out=a_out[:], in_=r_[:], func=AF.Sqrt)
            nc.vector.tensor_tensor(out=a_out[:], in0=g_sb[:], in1=a_out[:],
                                    op=ALU.mult)
            nc.vector.tensor_tensor(out=c_out[:], in0=m[:], in1=a_out[:],
                                    op=ALU.mult)
            nc.vector.tensor_tensor(out=c_out[:], in0=b_sb[:], in1=c_out[:],
                                    op=ALU.subtract)

        nchunks_r = [-(-nblkR[r] // GBLK) for r in range(2)]

        for l in range(DEPTH):
            # ---- payload: stream (l=0) or batched dma_gather (l=1)
            gtiles = [[], []]
            for r in range(2):
                for j in range(nchunks_r[r]):
                    nb = min(GBLK, nblkR[r] - GBLK * j)
                    gt = gp.tile([128, GBLK * D], F16, tag=f"G{r}",
                                 name=f"G{l}_{r}_{j}")
                    gtiles[r].append(gt)
                    if l == 0:
                        nc.sync.dma_start(
                            out=gt[:, :nb * D],
                            in_=pay_d[r][:, GBLK * j * D:(GBLK * j + nb) * D])
                    else:
                        view = vtab[0:BOUND, :] if r == 0 else vtab[BOUND:tok_n, :]
                        nidx = nb * BLK
                        nc.gpsimd.dma_gather(
                            gt[:, :nb * D].rearrange("p (b d) -> p b d", d=D),
                            view, idx_sb[r][:, GBLK * 8 * j:GBLK * 8 * j + nb * 8],
                            nidx, nidx, D,
                            queue_num=(r * nchunks_r[0] + j) % 4)
                nc_dummy = None  # noqa

            # ---- per-tile segment-matmul aggregation
            for t in range(nt):
                nbt = nsub_t[t]
                s0 = suboff[t]
                St = ck.tile([128, maxsub * 128], F16, tag="S", name=f"S{l}_{t}")
                dr = drel_sb[:, s0:s0 + nbt]
                dr_b = bass.AP(dr.tensor, dr.offset, dr.ap + [[0, 128]])
                nc.vector.tensor_tensor(
                    out=St[:, :nbt * 128].rearrange("p (b j) -> p b j", j=128),
                    in0=iota_sb[:, :nbt * 128].rearrange("p (b j) -> p b j", j=128),
                    in1=dr_b, op=ALU.is_equal)
                ps = psA.tile([128, D], F32, tag="agg", name=f"agg{l}_{t}")
                nc.tensor.matmul(ps[:], zeros16[:], zeros16[:],
                                 start=True, stop=False)
                si = 0
                for r in range(2):
                    b0, nb = int(blkoff[t][r]), int(nblk_t[t][r])
                    for bi in range(nb):
                        gb = b0 + bi
                        gt = gtiles[r][gb // GBLK]
                        slot = gb % GBLK
                        nc.tensor.matmul(
                            ps[:], St[:, si * 128:(si + 1) * 128],
                            gt[:, slot * D:(slot + 1) * D],
                            start=False, stop=(si == nbt - 1))
                        si += 1
                acc2 = sp.tile([128, D], F16, tag="acc2", name=f"acc2{l}_{t}")
                nc.vector.tensor_scalar_mul(out=acc2[:], in0=ps[:],
                                            scalar1=invdeg_sb[:, t:t + 1])
                pv = psT.tile([128, 128], F16, tag="tr", name=f"tr{l}_{t}")
                nc.tensor.transpose(pv[:], acc2[:], ident16[:])
                nc.scalar.activation(out=agg16[:, t * 128:(t + 1) * 128],
                                     in_=pv[:], func=AF.Copy)

            # ---- dense sweep 1: GCN linear + residual -> u; stats of u
            ssum1 = sp.tile([D, nch], F32, tag="ssum", name=f"ssum{l}1")
            ssq1 = sp.tile([D, nch], F32, tag="ssq", name=f"ssq{l}1")
            for ci, (c0, cw) in enumerate(chunks):
                sl = slice(c0, c0 + cw)
                ph = psG.tile([D, CHUNK], F32, tag="gcn", name=f"ph{l}{c0}")
                if l == 0:
                    nc.tensor.matmul(ph[:, :cw], wg_sb[0][:], agg16[:, sl],
                                     start=True, stop=True)
                else:
                    nc.tensor.matmul(ph[:, :cw], wg1p[:], agg16[:, sl],
                                     start=True, stop=False)
                    nc.tensor.matmul(ph[:, :cw], cw2_16[:], mask_sb[:, sl],
                                     start=False, stop=True)
                nc.vector.tensor_tensor(out=u16[:, sl], in0=ph[:, :cw],
                                        in1=xr16[:, sl], op=ALU.add)
                rw = max(0, min(cw, sh_real - c0))
                if rw == 0:
                    nc.vector.memset(ssum1[:, ci:ci + 1], 0.0)
                    nc.vector.memset(ssq1[:, ci:ci + 1], 0.0)
                    continue
                nc.vector.tensor_reduce(out=ssum1[:, ci:ci + 1],
                                        in_=u16[:, c0:c0 + rw],
                                        axis=mybir.AxisListType.X, op=ALU.add)
                sq = ck.tile([D, CHUNK], F16, tag="sq", name=f"sq{l}1{ci}")
                nc.scalar.activation(out=sq[:, :rw], in_=u16[:, c0:c0 + rw],
                                     func=AF.Square,
                                     accum_out=ssq1[:, ci:ci + 1])
            s2a = sp.tile([D, 2], F32, tag="s2", name=f"s2a{l}")
            nc.vector.tensor_reduce(out=s2a[:, 0:1], in_=ssum1[:],
                                    axis=mybir.AxisListType.X, op=ALU.add)
            nc.vector.tensor_reduce(out=s2a[:, 1:2], in_=ssq1[:],
                                    axis=mybir.AxisListType.X, op=ALU.add)
            a1 = sp.tile([D, 1], F32, tag="co", name=f"a1_{l}")
            c1 = sp.tile([D, 1], F32, tag="co", name=f"c1_{l}")
            bn_coeffs(l, 1, s2a, a1, c1)

            # ---- dense sweep 2: BN1 affine -> FF -> v; stats; (l=0) vshard
            ssum2 = sp.tile([D, nch], F32, tag="ssum", name=f"ssum{l}2")
            ssq2 = sp.tile([D, nch], F32, tag="ssq", name=f"ssq{l}2")
            for ci, (c0, cw) in enumerate(chunks):
                sl = slice(c0, c0 + cw)
                xp = ck.tile([D, CHUNK], F16, tag="xp", name=f"xp{l}{c0}")
                nc.vector.tensor_scalar(out=xp[:, :cw], in0=u16[:, sl],
                                        scalar1=a1[:], scalar2=c1[:],
                                        op0=ALU.mult, op1=ALU.add)
                py = psY.tile([D, CHUNK], F32, tag="ff2", name=f"py{l}{c0}")
                for h in range(H // D):
                    pr = psF.tile([D, CHUNK], F32, tag="ff1",
                                  name=f"pr{l}{c0}{h}")
                    nc.tensor.matmul(pr[:, :cw], w1_sb[l][:, h * D:(h + 1) * D],
                                     xp[:, :cw], start=True, stop=True)
                    rh = ck.tile([D, CHUNK], F16, tag="rh", name=f"rh{l}{c0}{h}")
                    nc.scalar.activation(out=rh[:, :cw], in_=pr[:, :cw],
                                         func=AF.Relu, bias=fb1_sb[l][:, h:h + 1],
                                         scale=1.0)
                    nc.tensor.matmul(py[:, :cw], w2_sb[l][h][:], rh[:, :cw],
                                     start=(h == 0), stop=(h == H // D - 1))
                nc.vector.tensor_tensor(out=v16[:, sl], in0=py[:, :cw],
                                        in1=xp[:, :cw], op=ALU.add)
                rw = max(0, min(cw, sh_real - c0))
                if l == 0 and rw < cw:
                    nc.vector.memset(v16[:, c0 + rw:c0 + cw], 0.0)
                if rw > 0:
                    nc.vector.tensor_reduce(out=ssum2[:, ci:ci + 1],
                                            in_=v16[:, c0:c0 + rw],
                                            axis=mybir.AxisListType.X,
                                            op=ALU.add)
                    sq = ck.tile([D, CHUNK], F16, tag="sq", name=f"sq{l}2{ci}")
                    nc.scalar.activation(out=sq[:, :rw], in_=v16[:, c0:c0 + rw],
                                         func=AF.Square,
                                         accum_out=ssq2[:, ci:ci + 1])
                else:
                    nc.vector.memset(ssum2[:, ci:ci + 1], 0.0)
                    nc.vector.memset(ssq2[:, ci:ci + 1], 0.0)
                if l == 0:
                    for t in range(c0 // 128, (c0 + cw) // 128):
                        pv = psT.tile([128, 128], F16, tag="tr", name=f"tv{t}")
                        nc.tensor.transpose(pv[:], v16[:, t * 128:(t + 1) * 128],
                                            ident16[:])
                        vT = sp.tile([128, D], F16, tag="vT", name=f"vT{t}")
                        nc.scalar.activation(out=vT[:], in_=pv[:], func=AF.Copy)
                        nc.sync.dma_start(out=vshard[t * 128:(t + 1) * 128, :],
                                          in_=vT[:])
            s2b = sp.tile([D, 2], F32, tag="s2", name=f"s2b{l}")
            nc.vector.tensor_reduce(out=s2b[:, 0:1], in_=ssum2[:],
                                    axis=mybir.AxisListType.X, op=ALU.add)
            nc.vector.tensor_reduce(out=s2b[:, 1:2], in_=ssq2[:],
                                    axis=mybir.AxisListType.X, op=ALU.add)

            if l == 0:
                nc.gpsimd.collective_compute(
                    "AllGather", ALU.bypass, replica_groups=rg,
                    ins=[vshard[:]], outs=[vtab[:]])
                a2 = sp.tile([D, 1], F32, tag="co", name="a2_0")
                c2 = sp.tile([D, 1], F32, tag="co", name="c2_0")
                bn_coeffs(l, 2, s2b, a2, c2)
                nc.vector.tensor_scalar_mul(out=wg1p[:], in0=wg_sb[1][:],
                                            scalar1=a2[:])
                c2_16 = sp.tile([D, 1], F16, tag="c216", name="c2_16")
                nc.vector.tensor_copy(out=c2_16[:], in_=c2[:])
                pcw = psG.tile([D, CHUNK], F32, tag="gcn", name="pcw2")
                nc.tensor.matmul(pcw[0:1, 0:D], c2_16[:], wg_sb[1][:],
                                 start=True, stop=True)
                nc.scalar.activation(out=cw2_16[:], in_=pcw[0:1, 0:D],
                                     func=AF.Copy)
                nc.vector.tensor_scalar(out=xr16[:], in0=v16[:],
                                        scalar1=a2[:], scalar2=c2[:],
                                        op0=ALU.mult, op1=ALU.add)
            else:
                a2p = sp.tile([D, 1], F32, tag="co", name="a2_1")
                c2p = sp.tile([D, 1], F32, tag="co", name="c2_1")
                bn_coeffs(l, 2, s2b, a2p, c2p)
                nc.vector.tensor_scalar_mul(out=clsw2[:], in0=clsw_sb[:],
                                            scalar1=a2p[:])
                c2p_16 = sp.tile([D, 1], F16, tag="c216", name="c2p_16")
                nc.vector.tensor_copy(out=c2p_16[:], in_=c2p[:])
                pcb = psY.tile([D, CHUNK], F32, tag="ff2", name="pcb")
                nc.tensor.matmul(pcb[0:16, 0:1], clsw_sb[:], c2p_16[:],
                                 start=True, stop=True)
                nc.vector.tensor_tensor(out=clsb2[:], in0=pcb[0:16, 0:1],
                                        in1=clsb_sb[:], op=ALU.add)
                for c0, cw in chunks:
                    sl = slice(c0, c0 + cw)
                    pc = psY.tile([D, CHUNK], F32, tag="ff2", name=f"pc{c0}")
                    nc.tensor.matmul(pc[0:16, :cw], clsw2[:], v16[:, sl],
                                     start=True, stop=True)
                    oc = ck.tile([16, CHUNK], F32, tag="oc", name=f"oc{c0}")
                    nc.scalar.activation(out=oc[:, :cw], in_=pc[0:16, :cw],
                                         func=AF.Identity, bias=clsb2[:],
                                         scale=1.0)
                    nc.sync.dma_start(out=out_d[:, sl], in_=oc[:, :cw])

    nc.compile()
    return nc


# ----------------------------------------------------------------------------
# Entry points
# ----------------------------------------------------------------------------

def _make_in_maps(cfg, inputs):
    W_gcn = np.asarray(inputs["W_gcn"], np.float32)
    ff_w1 = np.asarray(inputs["ff_w1"], np.float32)
    ff_b1 = np.asarray(inputs["ff_b1"], np.float32)
    ff_w2 = np.asarray(inputs["ff_w2"], np.float32)
    cls_w = np.asarray(inputs["cls_w"], np.float32)
    cls_b = np.asarray(inputs["cls_b"], np.float32)

    shared = {
        "clsw": np.ascontiguousarray(cls_w.astype(np.float16)),
        "clsb": np.ascontiguousarray(cls_b.reshape(16, 1)),
        "iotar": cfg["iota_rep"],
    }
    for l in range(DEPTH):
        shared[f"wg{l}"] = np.ascontiguousarray(W_gcn[l].astype(np.float16))
        shared[f"w1_{l}"] = np.ascontiguousarray(ff_w1[l].astype(np.float16))
        shared[f"fb1_{l}"] = np.ascontiguousarray(
            ff_b1[l].reshape(H // D, D).T)
        shared[f"w2_{l}"] = np.ascontiguousarray(ff_w2[l].astype(np.float16))
        shared[f"g1_{l}"] = np.ascontiguousarray(
            np.asarray(inputs["bn1_g"], np.float32)[l].reshape(D, 1))
        shared[f"b1_{l}"] = np.ascontiguousarray(
            np.asarray(inputs["bn1_b"], np.float32)[l].reshape(D, 1))
        shared[f"g2_{l}"] = np.ascontiguousarray(
            np.asarray(inputs["bn2_g"], np.float32)[l].reshape(D, 1))
        shared[f"b2_{l}"] = np.ascontiguousarray(
            np.asarray(inputs["bn2_b"], np.float32)[l].reshape(D, 1))

    sh = cfg["sh"]
    in_maps = []
    for c in range(CORES):
        m = dict(shared)
        m["x016"] = np.ascontiguousarray(
            cfg["table0"][c * sh:(c + 1) * sh].T.astype(np.float16))
        m["pay0"] = cfg["pay"][0][c]
        m["pay1"] = cfg["pay"][1][c]
        m["idx0"] = cfg["idxw"][0][c]
        m["idx1"] = cfg["idxw"][1][c]
        m["drel"] = np.ascontiguousarray(cfg["drel"][c])
        m["invdeg"] = np.ascontiguousarray(cfg["invdeg"][c])
        m["mask16"] = np.ascontiguousarray(cfg["mask"][c].reshape(1, sh))
        in_maps.append(m)
    return in_maps


def _postprocess(cfg, results):
    sh, sh_real = cfg["sh"], cfg["sh_real"]
    N = cfg["N"]
    node_of_tok = cfg["node_of_tok"]
    out = np.empty((N, 16), np.float32)
    for c in range(CORES):
        arr = results[c]["out_fm"]
        toks = np.arange(c * sh, c * sh + sh_real)
        out[node_of_tok[toks]] = arr.T[:sh_real]
    return out


def _ensure_axon_hooks():
    try:
        import antenv.axon_hooks  # noqa: F401
        return
    except ImportError:
        pass
    import types
    import antenv
    mod = types.ModuleType("antenv.axon_hooks")
    mod._hook = None

    def set_axon_ntff_profile_hook(h):
        mod._hook = h

    def get_axon_ntff_profile_hook():
        return mod._hook

    mod.set_axon_ntff_profile_hook = set_axon_ntff_profile_hook
    mod.get_axon_ntff_profile_hook = get_axon_ntff_profile_hook
    sys.modules["antenv.axon_hooks"] = mod
    antenv.axon_hooks = mod
    try:
        from trn_agent_boot.trn_boot import _ntff_profile_via_ctypes
        h = _ntff_profile_via_ctypes("/opt/axon/libaxon_pjrt.so")
        if h is not None:
            mod._hook = h
    except Exception as e:  # pragma: no cover
        print(f"ntff hook setup failed: {e}", file=sys.stderr)


_CACHE = {}


def run(trace=False, **inputs):
    if trace:
        _ensure_axon_hooks()
    nodes = np.asarray(inputs["nodes"], np.float32)
    edge_src = np.asarray(inputs["edge_src"], np.int64)
    edge_dst = np.asarray(inputs["edge_dst"], np.int64)
    cfg = _prepare(nodes, edge_src, edge_dst)

    key = (nodes.shape, len(edge_src), int(cfg["suboff"][-1]))
    if key not in _CACHE:
        _CACHE[key] = build_program(cfg)
    nc = _CACHE[key]

    in_maps = _make_in_maps(cfg, inputs)
    res = run_bass_kernel_spmd(nc, in_maps, list(range(CORES)), trace=trace)
    return _postprocess(cfg, res.results), res


def kernel(**inputs) -> np.ndarray:
    out, _ = run(trace=False, **inputs)
    return out


# revision 13
# speedup vs baseline: 2.2056x; 1.0211x over previous
"""Trainium2 Bass kernel for nn_NodeClassifier (gnn_message_passing).

Strategy (8 NeuronCores, SPMD):
  - Nodes block-partitioned by id across 8 cores (6250 each, padded to 6272).
  - Edges partitioned by dst core, grouped per 128-node dst tile, split into
    two src-token ranges (A: tok < 32768, B: rest) so gather indices fit
    int16, and padded to 128-edge blocks (block structure shared across
    cores = max over cores).
  - Aggregation is a segment-sum done ON THE TENSOR ENGINE: per 128-edge
    block, a [128 edges x 128 nodes] 0/1 selection matrix S (built on the
    vector engine from compact per-edge dst slots via iota==d) multiplies
    the gathered [128 edges x 128 feat] fp16 payload, accumulating into a
    per-tile PSUM [128 nodes x 128 feat]. A leading zero-matmul clears the
    accumulator so has_written semantics are never relied on.
  - Layer-0 payload is host-expanded (contiguous DMA streams). Layer-1
    payload is fetched with batched dma_gather (custom SWDGE instruction,
    1024 indices per call) from the AllGathered raw-v table.
  - All dense compute (GCN linear, BN, FF, cls) in fp16 matmuls,
    feature-major. BN stats via per-chunk DVE reduce + ACT Square accum_out,
    AllReduced (tiny). b_gcn dropped (BN(z+const)==BN(z), exact).
  - Layer-0 BN2 folded across the halo exchange: AllGather ships RAW v;
    a2 folds into layer-1's GCN weight, c2 via a rank-1 (c2^T W) x mask
    matmul and the local residual. Layer-1 BN2 folds into the classifier.
  - Weights replicated. Program identical on all cores.
"""

import os
import sys
import numpy as np

for _p in ("/opt/trn_rl_repo",):
    if _p not in sys.path and os.path.isdir(_p):
        sys.path.insert(0, _p)

from contextlib import ExitStack

import concourse.bass as bass
import concourse.bacc as bacc
import concourse.mybir as mybir
import concourse.tile as tile
from concourse.bass_utils import run_bass_kernel_spmd
from concourse.masks import make_identity

F32 = mybir.dt.float32
F16 = mybir.dt.float16
I16 = mybir.dt.int16
AF = mybir.ActivationFunctionType
ALU = mybir.AluOpType

CORES = 8
D = 128
H = 512
DEPTH = 2
EPS = 1e-5
CHUNK = 512
BOUND = 32768          # int16 gather-index range split
BLK = 128              # edges per gather/matmul block
GBLK = 8               # blocks per dma_gather (1024-descriptor ring cap)


# ----------------------------------------------------------------------------
# Host-side preparation
# ----------------------------------------------------------------------------

def _prepare(nodes, edge_src, edge_dst):
    N = nodes.shape[0]
    assert N % CORES == 0
    sh_real = N // CORES
    nt = -(-sh_real // 128)
    sh = nt * 128
    if sh == sh_real:
        nt += 1
        sh += 128
    tok_n = CORES * sh

    # permutation: per core block, sort nodes by degree ascending (keeps the
    # dense phase layout of the earlier kernel; not load-bearing here)
    deg = np.bincount(edge_dst, minlength=N).astype(np.int64)
    tok_of_node = np.empty(N, np.int64)
    node_of_tok = np.full(tok_n, -1, np.int64)
    for c in range(CORES):
        ids = np.arange(c * sh_real, (c + 1) * sh_real)
        order = np.argsort(deg[ids], kind="stable")
        toks = c * sh + np.arange(sh_real)
        tok_of_node[ids[order]] = toks
        node_of_tok[toks] = ids[order]

    dst_tok = tok_of_node[edge_dst]
    src_tok = tok_of_node[edge_src]

    e_core = dst_tok // sh
    e_slot = dst_tok % sh
    e_t = e_slot // 128
    e_p = e_slot % 128
    e_r = (src_tok >= BOUND).astype(np.int64)

    # per (core, tile, range) edge counts -> shared block counts
    cnt = np.zeros((CORES, nt, 2), np.int64)
    np.add.at(cnt, (e_core, e_t, e_r), 1)
    nblk_t = np.maximum(-(-cnt.max(axis=0) // BLK), 1)  # [nt, 2]
    blkoff = np.zeros((nt, 2), np.int64)
    nblkR = [0, 0]
    for r in range(2):
        off = 0
        for t in range(nt):
            blkoff[t, r] = off
            off += nblk_t[t, r]
        nblkR[r] = off

    # per-core edge placement: edge -> (range, global block, lane)
    # order within (core, tile, range): stable original order
    idx_arr = [np.zeros((CORES, nblkR[r] * BLK), np.int64) for r in range(2)]
    dloc = [np.full((CORES, nblkR[r] * BLK), 999.0, np.float32) for r in range(2)]
    paytok = [np.full((CORES, nblkR[r] * BLK), -1, np.int64) for r in range(2)]
    order = np.lexsort((np.arange(len(dst_tok)), e_r, e_t, e_core))
    # rank within (core, tile, range)
    key = ((e_core * nt + e_t) * 2 + e_r)
    ks = key[order]
    starts = np.searchsorted(ks, np.arange(CORES * nt * 2), side="left")
    rank = np.arange(len(order)) - starts[ks]
    ec, et, er, ep = e_core[order], e_t[order], e_r[order], e_p[order]
    st = src_tok[order]
    pos = (blkoff[et, er] * BLK + rank)
    for r in range(2):
        m = er == r
        idx_arr[r][ec[m], pos[m]] = st[m] - r * BOUND
        dloc[r][ec[m], pos[m]] = ep[m]
        paytok[r][ec[m], pos[m]] = st[m]

    # invdeg [128, nt] per core (0 for dummy slots)
    cnt_tok = np.bincount(dst_tok, minlength=tok_n)
    deg_tok = cnt_tok.reshape(CORES, sh)
    node_ok = node_of_tok.reshape(CORES, sh) >= 0
    iv = (1.0 / np.maximum(deg_tok, 1.0)) * node_ok
    mask = ((deg_tok > 0) & node_ok).astype(np.float16)
    invdeg = np.zeros((CORES, 128, nt), np.float32)
    for c in range(CORES):
        invdeg[c] = iv[c].reshape(nt, 128).T

    # replicated full node table [tok_n, D]
    table0 = np.zeros((tok_n, D), np.float32)
    real = node_of_tok >= 0
    table0[real] = nodes[node_of_tok[real]]
    t16 = table0.astype(np.float16)

    # layer-0 payload streams (edge-blocked x0 rows), fp16
    pay = []
    for r in range(2):
        p = np.zeros((CORES, nblkR[r] * BLK, D), np.float16)
        valid = paytok[r] >= 0
        p[valid] = t16[paytok[r][valid]]
        # gather layout: index i -> (partition i%128, block i//128)
        p = p.reshape(CORES, nblkR[r], BLK, D).transpose(0, 2, 1, 3)
        pay.append(np.ascontiguousarray(p.reshape(CORES, 128, nblkR[r] * D)))

    # wrapped int16 index arrays [128, nblkR*8] (replicated across 8 Q7 cores)
    idxw = []
    for r in range(2):
        w = idx_arr[r].reshape(CORES, -1, 16)  # [C, nblk*8, 16]
        w = w.transpose(0, 2, 1).astype(np.int16)  # [C, 16, nblk*8]
        idxw.append(np.ascontiguousarray(np.tile(w, (1, 8, 1))))

    # d_rel [128, nsub] fp16: per tile, its A-blocks then B-blocks
    # (lane -> partition)
    nsub_t = nblk_t.sum(axis=1)
    suboff = np.concatenate([[0], np.cumsum(nsub_t)])
    nsub = int(suboff[-1])
    drel = np.zeros((CORES, 128, nsub), np.float16)
    for t in range(nt):
        s0 = suboff[t]
        for r in range(2):
            b0, nb = blkoff[t, r], nblk_t[t, r]
            seg = dloc[r][:, b0 * BLK:(b0 + nb) * BLK].reshape(CORES, nb, BLK)
            drel[:, :, s0:s0 + nb] = seg.transpose(0, 2, 1).astype(np.float16)
            s0 += nb

    maxsub = int(nsub_t.max())
    iota_rep = np.tile(np.arange(128, dtype=np.float16), maxsub)[None, :]
    iota_rep = np.ascontiguousarray(np.broadcast_to(
        iota_rep, (128, maxsub * 128)))

    return dict(
        N=N, sh_real=sh_real, sh=sh, nt=nt, tok_n=tok_n,
        nblk_t=nblk_t, blkoff=blkoff, nblkR=nblkR,
        nsub_t=[int(x) for x in nsub_t], suboff=[int(x) for x in suboff],
        maxsub=maxsub, iota_rep=iota_rep,
        idxw=idxw, pay=pay, drel=drel,
        invdeg=invdeg, mask=mask, table0=table0, node_of_tok=node_of_tok,
    )


# ----------------------------------------------------------------------------
# Program builder
# ----------------------------------------------------------------------------

def build_program(cfg):
    nt, sh, sh_real = cfg["nt"], cfg["sh"], cfg["sh_real"]
    tok_n = cfg["tok_n"]
    nblk_t, blkoff, nblkR = cfg["nblk_t"], cfg["blkoff"], cfg["nblkR"]
    nsub_t, suboff, maxsub = cfg["nsub_t"], cfg["suboff"], cfg["maxsub"]
    N = cfg["N"]
    rg = [list(range(CORES))]

    chunks = []
    c0 = 0
    while c0 < sh:
        cw = min(CHUNK, sh - c0)
        chunks.append((c0, cw))
        c0 += cw
    nch = len(chunks)

    nc = bacc.Bacc("TRN2", target_bir_lowering=False, debug=False,
                   num_devices=CORES, num_swdge_queues=4)

    # ---- I/O declarations
    pay_d = [nc.dram_tensor(f"pay{r}", [128, nblkR[r] * D], F16,
                            kind="ExternalInput") for r in range(2)]
    idx_d = [nc.dram_tensor(f"idx{r}", [128, nblkR[r] * 8], I16,
                            kind="ExternalInput") for r in range(2)]
    drel_d = nc.dram_tensor("drel", [128, suboff[-1]], F16,
                            kind="ExternalInput")
    iota_d = nc.dram_tensor("iotar", [128, maxsub * 128], F16,
                            kind="ExternalInput")
    x0_d = nc.dram_tensor("x016", [D, sh], F16, kind="ExternalInput")
    invdeg_d = nc.dram_tensor("invdeg", [128, nt], F32, kind="ExternalInput")
    mask_d = nc.dram_tensor("mask16", [1, sh], F16, kind="ExternalInput")
    wg_d = [nc.dram_tensor(f"wg{l}", [D, D], F16, kind="ExternalInput")
            for l in range(DEPTH)]
    w1_d = [nc.dram_tensor(f"w1_{l}", [D, H], F16, kind="ExternalInput")
            for l in range(DEPTH)]
    fb1_d = [nc.dram_tensor(f"fb1_{l}", [D, H // D], F32, kind="ExternalInput")
             for l in range(DEPTH)]
    w2_d = [nc.dram_tensor(f"w2_{l}", [H, D], F16, kind="ExternalInput")
            for l in range(DEPTH)]
    bn_d = {}
    for l in range(DEPTH):
        for nm in ("g1", "b1", "g2", "b2"):
            bn_d[(nm, l)] = nc.dram_tensor(f"{nm}_{l}", [D, 1], F32,
                                           kind="ExternalInput")
    clsw_d = nc.dram_tensor("clsw", [D, 16], F16, kind="ExternalInput")
    clsb_d = nc.dram_tensor("clsb", [16, 1], F32, kind="ExternalInput")
    out_d = nc.dram_tensor("out_fm", [16, sh], F32, kind="ExternalOutput")

    with tile.TileContext(nc) as tc, ExitStack() as ctx:
        dram = ctx.enter_context(tc.tile_pool(name="dram", bufs=1, space="DRAM"))
        wp = ctx.enter_context(tc.tile_pool(name="weights", bufs=1))
        big = ctx.enter_context(tc.tile_pool(name="big", bufs=1))
        gp = ctx.enter_context(tc.tile_pool(name="gather", bufs=10))
        sp = ctx.enter_context(tc.tile_pool(name="small", bufs=4))
        ck = ctx.enter_context(tc.tile_pool(name="chunk", bufs=3))
        psA = ctx.enter_context(tc.tile_pool(name="psA", bufs=2, space="PSUM"))
        psG = ctx.enter_context(tc.tile_pool(name="psG", bufs=2, space="PSUM"))
        psF = ctx.enter_context(tc.tile_pool(name="psF", bufs=2, space="PSUM"))
        psY = ctx.enter_context(tc.tile_pool(name="psY", bufs=1, space="PSUM"))
        psT = ctx.enter_context(tc.tile_pool(name="psT", bufs=1, space="PSUM"))

        vshard = dram.tile([sh, D], F16, name="vshard")
        vtab = dram.tile([tok_n, D], F16, addr_space="Shared", name="vtab")
        bn_in, bn_out = {}, {}
        for l in range(DEPTH):
            for j in (1, 2):
                bn_in[(l, j)] = dram.tile([D, 2], F32, name=f"bni{l}{j}")
                bn_out[(l, j)] = dram.tile([D, 2], F32, addr_space="Shared",
                                           name=f"bno{l}{j}")

        def load(dt_, shape, src, name):
            t = wp.tile(shape, dt_, name=name)
            nc.sync.dma_start(out=t[:], in_=src)
            return t

        idx_sb = [load(I16, [128, nblkR[r] * 8], idx_d[r][:], f"idx_sb{r}")
                  for r in range(2)]
        drel_sb = load(F16, [128, suboff[-1]], drel_d[:], "drel_sb")
        iota_sb = load(F16, [128, maxsub * 128], iota_d[:], "iota_sb")
        invdeg_sb = load(F32, [128, nt], invdeg_d[:], "invdeg_sb")
        mask_sb = load(F16, [1, sh], mask_d[:], "mask_sb")
        wg_sb = [load(F16, [D, D], wg_d[l][:], f"wg_sb{l}") for l in range(DEPTH)]
        w1_sb = [load(F16, [D, H], w1_d[l][:], f"w1_sb{l}") for l in range(DEPTH)]
        fb1_sb = [load(F32, [D, H // D], fb1_d[l][:], f"fb1_sb{l}")
                  for l in range(DEPTH)]
        w2_sb = [[load(F16, [D, D], w2_d[l][h * D:(h + 1) * D, :], f"w2_sb{l}_{h}")
                  for h in range(H // D)] for l in range(DEPTH)]
        bn_sb = {k: load(F32, [D, 1], v[:], f"bn_{k[0]}_{k[1]}")
                 for k, v in bn_d.items()}
        clsw_sb = load(F16, [D, 16], clsw_d[:], "clsw_sb")
        clsb_sb = load(F32, [16, 1], clsb_d[:], "clsb_sb")

        ident16 = wp.tile([128, 128], F16, name="ident16")
        make_identity(nc, ident16[:])
        zeros16 = wp.tile([128, 128], F16, name="zeros16")
        nc.vector.memset(zeros16[:], 0.0)

        wg1p = wp.tile([D, D], F16, name="wg1p")
        cw2_16 = wp.tile([1, D], F16, name="cw2_16")
        clsw2 = wp.tile([D, 16], F16, name="clsw2")
        clsb2 = wp.tile([16, 1], F32, name="clsb2")

        agg16 = big.tile([D, sh], F16, name="agg16")
        u16 = big.tile([D, sh], F16, name="u16")
        v16 = big.tile([D, sh], F16, name="v16")
        xr16 = big.tile([D, sh], F16, name="xr16")
        nc.sync.dma_start(out=xr16[:], in_=x0_d[:])

        def bn_coeffs(l, j, s2, a_out, c_out):
            nc.sync.dma_start(out=bn_in[(l, j)][:], in_=s2[:])
            nc.gpsimd.collective_compute(
                "AllReduce", ALU.add, replica_groups=rg,
                ins=[bn_in[(l, j)][:]], outs=[bn_out[(l, j)][:]])
            sums = sp.tile([D, 2], F32, tag="sums", name=f"sums{l}{j}")
            nc.sync.dma_start(out=sums[:], in_=bn_out[(l, j)][:])
            g_sb = bn_sb[(f"g{j}", l)]
            b_sb = bn_sb[(f"b{j}", l)]
            m = sp.tile([D, 1], F32, tag="bnv", name="m")
            msq = sp.tile([D, 1], F32, tag="bnv", name="msq")
            var = sp.tile([D, 1], F32, tag="bnv", name="var")
            r_ = sp.tile([D, 1], F32, tag="bnv", name="r")
            nc.vector.tensor_scalar_mul(out=m[:], in0=sums[:, 0:1],
                                        scalar1=1.0 / N)
            nc.vector.tensor_scalar_mul(out=msq[:], in0=sums[:, 1:2],
                                        scalar1=1.0 / N)
            nc.vector.tensor_tensor(out=var[:], in0=m[:], in1=m[:], op=ALU.mult)
            nc.vector.tensor_tensor(out=var[:], in0=msq[:], in1=var[:],
                                    op=ALU.subtract)
            nc.vector.tensor_scalar_add(out=var[:], in0=var[:], scalar1=EPS)
            nc.vector.reciprocal(out=r_[:], in_=var[:])
            nc.scalar.activation(out=a_out[:], in_=r_[:], func=AF.Sqrt)
            nc.vector.tensor_tensor(out=a_out[:], in0=g_sb[:], in1=a_out[:],
                                    op=ALU.mult)
            nc.vector.tensor_tensor(out=c_out[:], in0=m[:], in1=a_out[:],
                                    op=ALU.mult)
            nc.vector.tensor_tensor(out=c_out[:], in0=b_sb[:], in1=c_out[:],
                                    op=ALU.subtract)

        nchunks_r = [-(-nblkR[r] // GBLK) for r in range(2)]

        for l in range(DEPTH):
            # ---- payload: stream (l=0) or batched dma_gather (l=1)
            gtiles = [[], []]
            for r in range(2):
                for j in range(nchunks_r[r]):
                    nb = min(GBLK, nblkR[r] - GBLK * j)
                    gt = gp.tile([128, GBLK * D], F16, tag=f"G{r}",
                                 name=f"G{l}_{r}_{j}")
                    gtiles[r].append(gt)
                    if l == 0:
                        nc.sync.dma_start(
                            out=gt[:, :nb * D],
                            in_=pay_d[r][:, GBLK * j * D:(GBLK * j + nb) * D])
                    else:
                        view = vtab[0:BOUND, :] if r == 0 else vtab[BOUND:tok_n, :]
                        nidx = nb * BLK
                        nc.gpsimd.dma_gather(
                            gt[:, :nb * D].rearrange("p (b d) -> p b d", d=D),
                            view, idx_sb[r][:, GBLK * 8 * j:GBLK * 8 * j + nb * 8],
                            nidx, nidx, D,
                            queue_num=(r * nchunks_r[0] + j) % 4)
                nc_dummy = None  # noqa

            # ---- per-tile segment-matmul aggregation
            for t in range(nt):
                nbt = nsub_t[t]
                s0 = suboff[t]
                St = ck.tile([128, maxsub * 128], F16, tag="S", name=f"S{l}_{t}")
                dr = drel_sb[:, s0:s0 + nbt]
                dr_b = bass.AP(dr.tensor, dr.offset, dr.ap + [[0, 128]])
                nc.vector.tensor_tensor(
                    out=St[:, :nbt * 128].rearrange("p (b j) -> p b j", j=128),
                    in0=iota_sb[:, :nbt * 128].rearrange("p (b j) -> p b j", j=128),
                    in1=dr_b, op=ALU.is_equal)
                ps = psA.tile([128, D], F32, tag="agg", name=f"agg{l}_{t}")
                nc.tensor.matmul(ps[:], zeros16[:], zeros16[:],
                                 start=True, stop=False)
                si = 0
                for r in range(2):
                    b0, nb = int(blkoff[t][r]), int(nblk_t[t][r])
                    for bi in range(nb):
                        gb = b0 + bi
                        gt = gtiles[r][gb // GBLK]
                        slot = gb % GBLK
                        nc.tensor.matmul(
                            ps[:], St[:, si * 128:(si + 1) * 128],
                            gt[:, slot * D:(slot + 1) * D],
                            start=False, stop=(si == nbt - 1))
                        si += 1
                acc2 = sp.tile([128, D], F16, tag="acc2", name=f"acc2{l}_{t}")
                nc.vector.tensor_scalar_mul(out=acc2[:], in0=ps[:],
                                            scalar1=invdeg_sb[:, t:t + 1])
                pv = psT.tile([128, 128], F16, tag="tr", name=f"tr{l}_{t}")
                nc.tensor.transpose(pv[:], acc2[:], ident16[:])
                nc.scalar.activation(out=agg16[:, t * 128:(t + 1) * 128],
                                     in_=pv[:], func=AF.Copy)

            # ---- dense sweep 1: GCN linear + residual -> u; stats of u
            ssum1 = sp.tile([D, nch], F32, tag="ssum", name=f"ssum{l}1")
            ssq1 = sp.tile([D, nch], F32, tag="ssq", name=f"ssq{l}1")
            for ci, (c0, cw) in enumerate(chunks):
                sl = slice(c0, c0 + cw)
                ph = psG.tile([D, CHUNK], F32, tag="gcn", name=f"ph{l}{c0}")
                if l == 0:
                    nc.tensor.matmul(ph[:, :cw], wg_sb[0][:], agg16[:, sl],
                                     start=True, stop=True)
                else:
                    nc.tensor.matmul(ph[:, :cw], wg1p[:], agg16[:, sl],
                                     start=True, stop=False)
                    nc.tensor.matmul(ph[:, :cw], cw2_16[:], mask_sb[:, sl],
                                     start=False, stop=True)
                nc.vector.tensor_tensor(out=u16[:, sl], in0=ph[:, :cw],
                                        in1=xr16[:, sl], op=ALU.add)
                rw = max(0, min(cw, sh_real - c0))
                if rw == 0:
                    nc.vector.memset(ssum1[:, ci:ci + 1], 0.0)
                    nc.vector.memset(ssq1[:, ci:ci + 1], 0.0)
                    continue
                nc.vector.tensor_reduce(out=ssum1[:, ci:ci + 1],
                                        in_=u16[:, c0:c0 + rw],
                                        axis=mybir.AxisListType.X, op=ALU.add)
                sq = ck.tile([D, CHUNK], F16, tag="sq", name=f"sq{l}1{ci}")
                nc.scalar.activation(out=sq[:, :rw], in_=u16[:, c0:c0 + rw],
                                     func=AF.Square,
                                     accum_out=ssq1[:, ci:ci + 1])
            s2a = sp.tile([D, 2], F32, tag="s2", name=f"s2a{l}")
            nc.vector.tensor_reduce(out=s2a[:, 0:1], in_=ssum1[:],
                                    axis=mybir.AxisListType.X, op=ALU.add)
            nc.vector.tensor_reduce(out=s2a[:, 1:2], in_=ssq1[:],
                                    axis=mybir.AxisListType.X, op=ALU.add)
            a1 = sp.tile([D, 1], F32, tag="co", name=f"a1_{l}")
            c1 = sp.tile([D, 1], F32, tag="co", name=f"c1_{l}")
            bn_coeffs(l, 1, s2a, a1, c1)

            # ---- dense sweep 2: BN1 affine -> FF -> v; stats; (l=0) vshard
            ssum2 = sp.tile([D, nch], F32, tag="ssum", name=f"ssum{l}2")
            ssq2 = sp.tile([D, nch], F32, tag="ssq", name=f"ssq{l}2")
            for ci, (c0, cw) in enumerate(chunks):
                sl = slice(c0, c0 + cw)
                xp = ck.tile([D, CHUNK], F16, tag="xp", name=f"xp{l}{c0}")
                nc.vector.tensor_scalar(out=xp[:, :cw], in0=u16[:, sl],
                                        scalar1=a1[:], scalar2=c1[:],
                                        op0=ALU.mult, op1=ALU.add)
                py = psY.tile([D, CHUNK], F32, tag="ff2", name=f"py{l}{c0}")
                for h in range(H // D):
                    pr = psF.tile([D, CHUNK], F32, tag="ff1",
                                  name=f"pr{l}{c0}{h}")
                    nc.tensor.matmul(pr[:, :cw], w1_sb[l][:, h * D:(h + 1) * D],
                                     xp[:, :cw], start=True, stop=True)
                    rh = ck.tile([D, CHUNK], F16, tag="rh", name=f"rh{l}{c0}{h}")
                    nc.scalar.activation(out=rh[:, :cw], in_=pr[:, :cw],
                                         func=AF.Relu, bias=fb1_sb[l][:, h:h + 1],
                                         scale=1.0)
                    nc.tensor.matmul(py[:, :cw], w2_sb[l][h][:], rh[:, :cw],
                                     start=(h == 0), stop=(h == H // D - 1))
                nc.vector.tensor_tensor(out=v16[:, sl], in0=py[:, :cw],
                                        in1=xp[:, :cw], op=ALU.add)
                rw = max(0, min(cw, sh_real - c0))
                if l == 0 and rw < cw:
                    nc.vector.memset(v16[:, c0 + rw:c0 + cw], 0.0)
                if rw > 0:
                    nc.vector.tensor_reduce(out=ssum2[:, ci:ci + 1],
                                            in_=v16[:, c0:c0 + rw],
                                            axis=mybir.AxisListType.X,
                                            op=ALU.add)
                    sq = ck.tile([D, CHUNK], F16, tag="sq", name=f"sq{l}2{ci}")
                    nc.scalar.activation(out=sq[:, :rw], in_=v16[:, c0:c0 + rw],
                                         func=AF.Square,
                                         accum_out=ssq2[:, ci:ci + 1])
                else:
                    nc.vector.memset(ssum2[:, ci:ci + 1], 0.0)
                    nc.vector.memset(ssq2[:, ci:ci + 1], 0.0)
                if l == 0:
                    for t in range(c0 // 128, (c0 + cw) // 128):
                        pv = psT.tile([128, 128], F16, tag="tr", name=f"tv{t}")
                        nc.tensor.transpose(pv[:], v16[:, t * 128:(t + 1) * 128],
                                            ident16[:])
                        vT = sp.tile([128, D], F16, tag="vT", name=f"vT{t}")
                        nc.scalar.activation(out=vT[:], in_=pv[:], func=AF.Copy)
                        nc.sync.dma_start(out=vshard[t * 128:(t + 1) * 128, :],
                                          in_=vT[:])
            s2b = sp.tile([D, 2], F32, tag="s2", name=f"s2b{l}")
            nc.vector.tensor_reduce(out=s2b[:, 0:1], in_=ssum2[:],
                                    axis=mybir.AxisListType.X, op=ALU.add)
            nc.vector.tensor_reduce(out=s2b[:, 1:2], in_=ssq2[:],
                                    axis=mybir.AxisListType.X, op=ALU.add)

            if l == 0:
                nc.gpsimd.collective_compute(
                    "AllGather", ALU.bypass, replica_groups=rg,
                    ins=[vshard[:]], outs=[vtab[:]])
                a2 = sp.tile([D, 1], F32, tag="co", name="a2_0")
                c2 = sp.tile([D, 1], F32, tag="co", name="c2_0")
                bn_coeffs(l, 2, s2b, a2, c2)
                nc.vector.tensor_scalar_mul(out=wg1p[:], in0=wg_sb[1][:],
                                            scalar1=a2[:])
                c2_16 = sp.tile([D, 1], F16, tag="c216", name="c2_16")
                nc.vector.tensor_copy(out=c2_16[:], in_=c2[:])
                pcw = psG.tile([D, CHUNK], F32, tag="gcn", name="pcw2")
                nc.tensor.matmul(pcw[0:1, 0:D], c2_16[:], wg_sb[1][:],
                                 start=True, stop=True)
                nc.scalar.activation(out=cw2_16[:], in_=pcw[0:1, 0:D],
                                     func=AF.Copy)
                nc.vector.tensor_scalar(out=xr16[:], in0=v16[:],
                                        scalar1=a2[:], scalar2=c2[:],
                                        op0=ALU.mult, op1=ALU.add)
            else:
                a2p = sp.tile([D, 1], F32, tag="co", name="a2_1")
                c2p = sp.tile([D, 1], F32, tag="co", name="c2_1")
                bn_coeffs(l, 2, s2b, a2p, c2p)
                nc.vector.tensor_scalar_mul(out=clsw2[:], in0=clsw_sb[:],
                                            scalar1=a2p[:])
                c2p_16 = sp.tile([D, 1], F16, tag="c216", name="c2p_16")
                nc.vector.tensor_copy(out=c2p_16[:], in_=c2p[:])
                pcb = psY.tile([D, CHUNK], F32, tag="ff2", name="pcb")
                nc.tensor.matmul(pcb[0:16, 0:1], clsw_sb[:], c2p_16[:],
                                 start=True, stop=True)
                nc.vector.tensor_tensor(out=clsb2[:], in0=pcb[0:16, 0:1],
                                        in1=clsb_sb[:], op=ALU.add)
                for c0, cw in chunks:
                    sl = slice(c0, c0 + cw)
                    pc = psY.tile([D, CHUNK], F32, tag="ff2", name=f"pc{c0}")
                    nc.tensor.matmul(pc[0:16, :cw], clsw2[:], v16[:, sl],
                                     start=True, stop=True)
                    oc = ck.tile([16, CHUNK], F32, tag="oc", name=f"oc{c0}")
                    nc.scalar.activation(out=oc[:, :cw], in_=pc[0:16, :cw],
                                         func=AF.Identity, bias=clsb2[:],
                                         scale=1.0)
                    nc.sync.dma_start(out=out_d[:, sl], in_=oc[:, :cw])

    nc.compile()
    return nc


# ----------------------------------------------------------------------------
# Entry points
# ----------------------------------------------------------------------------

def _make_in_maps(cfg, inputs):
    W_gcn = np.asarray(inputs["W_gcn"], np.float32)
    ff_w1 = np.asarray(inputs["ff_w1"], np.float32)
    ff_b1 = np.asarray(inputs["ff_b1"], np.float32)
    ff_w2 = np.asarray(inputs["ff_w2"], np.float32)
    cls_w = np.asarray(inputs["cls_w"], np.float32)
    cls_b = np.asarray(inputs["cls_b"], np.float32)

    shared = {
        "clsw": np.ascontiguousarray(cls_w.astype(np.float16)),
        "clsb": np.ascontiguousarray(cls_b.reshape(16, 1)),
        "iotar": cfg["iota_rep"],
    }
    for l in range(DEPTH):
        shared[f"wg{l}"] = np.ascontiguousarray(W_gcn[l].astype(np.float16))
        shared[f"w1_{l}"] = np.ascontiguousarray(ff_w1[l].astype(np.float16))
        shared[f"fb1_{l}"] = np.ascontiguousarray(
            ff_b1[l].reshape(H // D, D).T)
        shared[f"w2_{l}"] = np.ascontiguousarray(ff_w2[l].astype(np.float16))
        shared[f"g1_{l}"] = np.ascontiguousarray(
            np.asarray(inputs["bn1_g"], np.float32)[l].reshape(D, 1))
        shared[f"b1_{l}"] = np.ascontiguousarray(
            np.asarray(inputs["bn1_b"], np.float32)[l].reshape(D, 1))
        shared[f"g2_{l}"] = np.ascontiguousarray(
            np.asarray(inputs["bn2_g"], np.float32)[l].reshape(D, 1))
        shared[f"b2_{l}"] = np.ascontiguousarray(
            np.asarray(inputs["bn2_b"], np.float32)[l].reshape(D, 1))

    sh = cfg["sh"]
    in_maps = []
    for c in range(CORES):
        m = dict(shared)
        m["x016"] = np.ascontiguousarray(
            cfg["table0"][c * sh:(c + 1) * sh].T.astype(np.float16))
        m["pay0"] = cfg["pay"][0][c]
        m["pay1"] = cfg["pay"][1][c]
        m["idx0"] = cfg["idxw"][0][c]
        m["idx1"] = cfg["idxw"][1][c]
        m["drel"] = np.ascontiguousarray(cfg["drel"][c])
        m["invdeg"] = np.ascontiguousarray(cfg["invdeg"][c])
        m["mask16"] = np.ascontiguousarray(cfg["mask"][c].reshape(1, sh))
        in_maps.append(m)
    return in_maps


def _postprocess(cfg, results):
    sh, sh_real = cfg["sh"], cfg["sh_real"]
    N = cfg["N"]
    node_of_tok = cfg["node_of_tok"]
    out = np.empty((N, 16), np.float32)
    for c in range(CORES):
        arr = results[c]["out_fm"]
        toks = np.arange(c * sh, c * sh + sh_real)
        out[node_of_tok[toks]] = arr.T[:sh_real]
    return out


def _ensure_axon_hooks():
    try:
        import antenv.axon_hooks  # noqa: F401
        return
    except ImportError:
        pass
    import types
    import antenv
    mod = types.ModuleType("antenv.axon_hooks")
    mod._hook = None

    def set_axon_ntff_profile_hook(h):
        mod._hook = h

    def get_axon_ntff_profile_hook():
        return mod._hook

    mod.set_axon_ntff_profile_hook = set_axon_ntff_profile_hook
    mod.get_axon_ntff_profile_hook = get_axon_ntff_profile_hook
    sys.modules["antenv.axon_hooks"] = mod
    antenv.axon_hooks = mod
    try:
        from trn_agent_boot.trn_boot import _ntff_profile_via_ctypes
        h = _ntff_profile_via_ctypes("/opt/axon/libaxon_pjrt.so")
        if h is not None:
            mod._hook = h
    except Exception as e:  # pragma: no cover
        print(f"ntff hook setup failed: {e}", file=sys.stderr)


_CACHE = {}


def run(trace=False, **inputs):
    if trace:
        _ensure_axon_hooks()
    nodes = np.asarray(inputs["nodes"], np.float32)
    edge_src = np.asarray(inputs["edge_src"], np.int64)
    edge_dst = np.asarray(inputs["edge_dst"], np.int64)
    cfg = _prepare(nodes, edge_src, edge_dst)

    key = (nodes.shape, len(edge_src), int(cfg["suboff"][-1]))
    if key not in _CACHE:
        _CACHE[key] = build_program(cfg)
    nc = _CACHE[key]

    in_maps = _make_in_maps(cfg, inputs)
    res = run_bass_kernel_spmd(nc, in_maps, list(range(CORES)), trace=trace)
    return _postprocess(cfg, res.results), res


def kernel(**inputs) -> np.ndarray:
    out, _ = run(trace=False, **inputs)
    return out


# revision 15
# speedup vs baseline: 2.2263x; 1.0094x over previous
"""Trainium2 Bass kernel for nn_NodeClassifier (gnn_message_passing).

Strategy (8 NeuronCores, SPMD):
  - Nodes block-partitioned by id across 8 cores (6250 each, padded to 6272).
  - Edges partitioned by dst core, grouped per 128-node dst tile, split into
    two src-token ranges (A: tok < 32768, B: rest) so gather indices fit
    int16, and padded to 128-edge blocks (block structure shared across
    cores = max over cores).
  - Aggregation is a segment-sum done ON THE TENSOR ENGINE: per 128-edge
    block, a [128 edges x 128 nodes] 0/1 selection matrix S (built on the
    vector engine from compact per-edge dst slots via iota==d) multiplies
    the gathered [128 edges x 128 feat] fp16 payload, accumulating into a
    per-tile PSUM [128 nodes x 128 feat]. A leading zero-matmul clears the
    accumulator so has_written semantics are never relied on.
  - Layer-0 payload is host-expanded (contiguous DMA streams). Layer-1
    payload is fetched with batched dma_gather (custom SWDGE instruction,
    1024 indices per call) from the AllGathered raw-v table.
  - All dense compute (GCN linear, BN, FF, cls) in fp16 matmuls,
    feature-major. BN stats via per-chunk DVE reduce + ACT Square accum_out,
    AllReduced (tiny). b_gcn dropped (BN(z+const)==BN(z), exact).
  - Layer-0 BN2 folded across the halo exchange: AllGather ships RAW v;
    a2 folds into layer-1's GCN weight, c2 via a rank-1 (c2^T W) x mask
    matmul and the local residual. Layer-1 BN2 folds into the classifier.
  - Weights replicated. Program identical on all cores.
"""

import os
import sys
import numpy as np

for _p in ("/opt/trn_rl_repo",):
    if _p not in sys.path and os.path.isdir(_p):
        sys.path.insert(0, _p)

from contextlib import ExitStack

import concourse.bass as bass
import concourse.bacc as bacc
import concourse.mybir as mybir
import concourse.tile as tile
from concourse.bass_utils import run_bass_kernel_spmd

F32 = mybir.dt.float32
F16 = mybir.dt.float16
I16 = mybir.dt.int16
AF = mybir.ActivationFunctionType
ALU = mybir.AluOpType

CORES = 8
D = 128
H = 512
DEPTH = 2
EPS = 1e-5
CHUNK = 512
H1 = 3072              # per-core row split: half 1 rows [0,3072)
H2 = 3200              # half 2 rows [3072,6272); both halves fit int16
BLK = 128              # edges per gather/matmul block
GBLK = 8               # blocks per dma_gather (1024-descriptor ring cap)


# ----------------------------------------------------------------------------
# Host-side preparation
# ----------------------------------------------------------------------------

def _prepare(nodes, edge_src, edge_dst):
    N = nodes.shape[0]
    assert N % CORES == 0
    sh_real = N // CORES
    nt = -(-sh_real // 128)
    sh = nt * 128
    if sh == sh_real:
        nt += 1
        sh += 128
    tok_n = CORES * sh

    # permutation: per core block, sort nodes by degree ascending (keeps the
    # dense phase layout of the earlier kernel; not load-bearing here)
    deg = np.bincount(edge_dst, minlength=N).astype(np.int64)
    tok_of_node = np.empty(N, np.int64)
    node_of_tok = np.full(tok_n, -1, np.int64)
    for c in range(CORES):
        ids = np.arange(c * sh_real, (c + 1) * sh_real)
        order = np.argsort(deg[ids], kind="stable")
        toks = c * sh + np.arange(sh_real)
        tok_of_node[ids[order]] = toks
        node_of_tok[toks] = ids[order]

    dst_tok = tok_of_node[edge_dst]
    src_tok = tok_of_node[edge_src]

    e_core = dst_tok // sh
    e_slot = dst_tok % sh
    e_t = e_slot // 128
    e_p = e_slot % 128
    s_core = src_tok // sh
    s_slot = src_tok % sh
    e_r = (s_slot >= H1).astype(np.int64)

    # per (core, tile, range) edge counts -> shared block counts
    cnt = np.zeros((CORES, nt, 2), np.int64)
    np.add.at(cnt, (e_core, e_t, e_r), 1)
    nblk_t = np.maximum(-(-cnt.max(axis=0) // BLK), 1)  # [nt, 2]
    blkoff = np.zeros((nt, 2), np.int64)
    nblkR = [0, 0]
    for r in range(2):
        off = 0
        for t in range(nt):
            blkoff[t, r] = off
            off += nblk_t[t, r]
        nblkR[r] = off

    # per-core edge placement: edge -> (range, global block, lane)
    # order within (core, tile, range): stable original order
    idx_arr = [np.zeros((CORES, nblkR[r] * BLK), np.int64) for r in range(2)]
    dloc = [np.full((CORES, nblkR[r] * BLK), 999.0, np.float32) for r in range(2)]
    paytok = [np.full((CORES, nblkR[r] * BLK), -1, np.int64) for r in range(2)]
    order = np.lexsort((np.arange(len(dst_tok)), e_r, e_t, e_core))
    # rank within (core, tile, range)
    key = ((e_core * nt + e_t) * 2 + e_r)
    ks = key[order]
    starts = np.searchsorted(ks, np.arange(CORES * nt * 2), side="left")
    rank = np.arange(len(order)) - starts[ks]
    ec, et, er, ep = e_core[order], e_t[order], e_r[order], e_p[order]
    st = src_tok[order]
    sc, ss = s_core[order], s_slot[order]
    row_r = [sc * H1 + ss, sc * H2 + (ss - H1)]
    pos = (blkoff[et, er] * BLK + rank)
    for r in range(2):
        m = er == r
        idx_arr[r][ec[m], pos[m]] = row_r[r][m]
        dloc[r][ec[m], pos[m]] = ep[m]
        paytok[r][ec[m], pos[m]] = st[m]

    # invdeg [128, nt] per core (0 for dummy slots)
    cnt_tok = np.bincount(dst_tok, minlength=tok_n)
    deg_tok = cnt_tok.reshape(CORES, sh)
    node_ok = node_of_tok.reshape(CORES, sh) >= 0
    iv = (1.0 / np.maximum(deg_tok, 1.0)) * node_ok
    mask = ((deg_tok > 0) & node_ok).astype(np.float16)
    invdeg = np.zeros((CORES, 128, nt), np.float32)
    for c in range(CORES):
        invdeg[c] = iv[c].reshape(nt, 128).T

    # replicated full node table [tok_n, D]
    table0 = np.zeros((tok_n, D), np.float32)
    real = node_of_tok >= 0
    table0[real] = nodes[node_of_tok[real]]
    t16 = table0.astype(np.float16)

    # layer-0 payload streams (edge-blocked x0 rows), fp16
    pay = []
    for r in range(2):
        p = np.zeros((CORES, nblkR[r] * BLK, D), np.float16)
        valid = paytok[r] >= 0
        p[valid] = t16[paytok[r][valid]]
        # gather layout: index i -> (partition i%128, block i//128)
        p = p.reshape(CORES, nblkR[r], BLK, D).transpose(0, 2, 1, 3)
        pay.append(np.ascontiguousarray(p.reshape(CORES, 128, nblkR[r] * D)))

    # wrapped int16 index arrays [128, nblkR*8] (replicated across 8 Q7 cores)
    idxw = []
    for r in range(2):
        w = idx_arr[r].reshape(CORES, -1, 16)  # [C, nblk*8, 16]
        w = w.transpose(0, 2, 1).astype(np.int16)  # [C, 16, nblk*8]
        idxw.append(np.ascontiguousarray(np.tile(w, (1, 8, 1))))

    # d_rel [128, nsub] fp16: per tile, its A-blocks then B-blocks
    # (lane -> partition)
    nsub_t = nblk_t.sum(axis=1)
    suboff = np.concatenate([[0], np.cumsum(nsub_t)])
    nsub = int(suboff[-1])
    drel = np.zeros((CORES, 128, nsub), np.float16)
    for t in range(nt):
        s0 = suboff[t]
        for r in range(2):
            b0, nb = blkoff[t, r], nblk_t[t, r]
            seg = dloc[r][:, b0 * BLK:(b0 + nb) * BLK].reshape(CORES, nb, BLK)
            drel[:, :, s0:s0 + nb] = seg.transpose(0, 2, 1).astype(np.float16)
            s0 += nb

    maxsub = int(nsub_t.max())
    iota_rep = np.tile(np.arange(128, dtype=np.float16), maxsub)[None, :]
    iota_rep = np.ascontiguousarray(np.broadcast_to(
        iota_rep, (128, maxsub * 128)))

    return dict(
        N=N, sh_real=sh_real, sh=sh, nt=nt, tok_n=tok_n,
        nblk_t=nblk_t, blkoff=blkoff, nblkR=nblkR,
        nsub_t=[int(x) for x in nsub_t], suboff=[int(x) for x in suboff],
        maxsub=maxsub, iota_rep=iota_rep,
        idxw=idxw, pay=pay, drel=drel,
        invdeg=invdeg, mask=mask, table0=table0, node_of_tok=node_of_tok,
    )


# ----------------------------------------------------------------------------
# Program builder
# ----------------------------------------------------------------------------

def build_program(cfg):
    nt, sh, sh_real = cfg["nt"], cfg["sh"], cfg["sh_real"]
    tok_n = cfg["tok_n"]
    nblk_t, blkoff, nblkR = cfg["nblk_t"], cfg["blkoff"], cfg["nblkR"]
    nsub_t, suboff, maxsub = cfg["nsub_t"], cfg["suboff"], cfg["maxsub"]
    N = cfg["N"]
    rg = [list(range(CORES))]

    chunks = []
    c0 = 0
    while c0 < sh:
        cw = min(CHUNK, sh - c0)
        chunks.append((c0, cw))
        c0 += cw
    nch = len(chunks)

    nc = bacc.Bacc("TRN2", target_bir_lowering=False, debug=False,
                   num_devices=CORES, num_swdge_queues=4)

    # ---- I/O declarations
    pay_d = [nc.dram_tensor(f"pay{r}", [128, nblkR[r] * D], F16,
                            kind="ExternalInput") for r in range(2)]
    idx_d = [nc.dram_tensor(f"idx{r}", [128, nblkR[r] * 8], I16,
                            kind="ExternalInput") for r in range(2)]
    drel_d = nc.dram_tensor("drel", [128, suboff[-1]], F16,
                            kind="ExternalInput")
    iota_d = nc.dram_tensor("iotar", [128, maxsub * 128], F16,
                            kind="ExternalInput")
    x0_d = nc.dram_tensor("x016", [D, sh], F16, kind="ExternalInput")
    invdeg_d = nc.dram_tensor("invdeg", [128, nt], F32, kind="ExternalInput")
    mask_d = nc.dram_tensor("mask16", [1, sh], F16, kind="ExternalInput")
    wg_d = [nc.dram_tensor(f"wg{l}", [D, D], F16, kind="ExternalInput")
            for l in range(DEPTH)]
    w1_d = [nc.dram_tensor(f"w1_{l}", [D, H], F16, kind="ExternalInput")
            for l in range(DEPTH)]
    fb1_d = [nc.dram_tensor(f"fb1_{l}", [D, H // D], F32, kind="ExternalInput")
             for l in range(DEPTH)]
    w2_d = [nc.dram_tensor(f"w2_{l}", [H, D], F16, kind="ExternalInput")
            for l in range(DEPTH)]
    bn_d = {}
    for l in range(DEPTH):
        for nm in ("g1", "b1", "g2", "b2"):
            bn_d[(nm, l)] = nc.dram_tensor(f"{nm}_{l}", [D, 1], F32,
                                           kind="ExternalInput")
    ident_d = nc.dram_tensor("ident16", [128, 128], F16, kind="ExternalInput")
    clsw_d = nc.dram_tensor("clsw", [D, 16], F16, kind="ExternalInput")
    clsb_d = nc.dram_tensor("clsb", [16, 1], F32, kind="ExternalInput")
    out_d = nc.dram_tensor("out_fm", [16, sh], F32, kind="ExternalOutput")

    with tile.TileContext(nc) as tc, ExitStack() as ctx:
        dram = ctx.enter_context(tc.tile_pool(name="dram", bufs=1, space="DRAM"))
        wp = ctx.enter_context(tc.tile_pool(name="weights", bufs=1))
        big = ctx.enter_context(tc.tile_pool(name="big", bufs=1))
        gp = ctx.enter_context(tc.tile_pool(name="gather", bufs=10))
        sp = ctx.enter_context(tc.tile_pool(name="small", bufs=4))
        ck = ctx.enter_context(tc.tile_pool(name="chunk", bufs=3))
        psA = ctx.enter_context(tc.tile_pool(name="psA", bufs=2, space="PSUM"))
        psG = ctx.enter_context(tc.tile_pool(name="psG", bufs=2, space="PSUM"))
        psF = ctx.enter_context(tc.tile_pool(name="psF", bufs=2, space="PSUM"))
        psY = ctx.enter_context(tc.tile_pool(name="psY", bufs=1, space="PSUM"))
        psT = ctx.enter_context(tc.tile_pool(name="psT", bufs=1, space="PSUM"))

        vshard = dram.tile([sh, D], F16, name="vshard")
        vtabH1 = dram.tile([CORES * H1, D], F16, addr_space="Shared",
                           name="vtabH1")
        vtabH2 = dram.tile([CORES * H2, D], F16, addr_space="Shared",
                           name="vtabH2")
        bn_in, bn_out = {}, {}
        for l in range(DEPTH):
            for j in (1, 2):
                bn_in[(l, j)] = dram.tile([D, 2], F32, name=f"bni{l}{j}")
                bn_out[(l, j)] = dram.tile([D, 2], F32, addr_space="Shared",
                                           name=f"bno{l}{j}")

        def load(dt_, shape, src, name):
            t = wp.tile(shape, dt_, name=name)
            nc.sync.dma_start(out=t[:], in_=src)
            return t

        idx_sb = [load(I16, [128, nblkR[r] * 8], idx_d[r][:], f"idx_sb{r}")
                  for r in range(2)]
        drel_sb = load(F16, [128, suboff[-1]], drel_d[:], "drel_sb")
        iota_sb = load(F16, [128, maxsub * 128], iota_d[:], "iota_sb")
        invdeg_sb = load(F32, [128, nt], invdeg_d[:], "invdeg_sb")
        mask_sb = load(F16, [1, sh], mask_d[:], "mask_sb")
        wg_sb = [load(F16, [D, D], wg_d[l][:], f"wg_sb{l}") for l in range(DEPTH)]
        w1_sb = [load(F16, [D, H], w1_d[l][:], f"w1_sb{l}") for l in range(DEPTH)]
        fb1_sb = [load(F32, [D, H // D], fb1_d[l][:], f"fb1_sb{l}")
                  for l in range(DEPTH)]
        w2_sb = [[load(F16, [D, D], w2_d[l][h * D:(h + 1) * D, :], f"w2_sb{l}_{h}")
                  for h in range(H // D)] for l in range(DEPTH)]
        bn_sb = {k: load(F32, [D, 1], v[:], f"bn_{k[0]}_{k[1]}")
                 for k, v in bn_d.items()}
        clsw_sb = load(F16, [D, 16], clsw_d[:], "clsw_sb")
        clsb_sb = load(F32, [16, 1], clsb_d[:], "clsb_sb")

        ident16 = load(F16, [128, 128], ident_d[:], "ident16")

        wg1p = wp.tile([D, D], F16, name="wg1p")
        cw2_16 = wp.tile([1, D], F16, name="cw2_16")
        clsw2 = wp.tile([D, 16], F16, name="clsw2")
        clsb2 = wp.tile([16, 1], F32, name="clsb2")

        agg16 = big.tile([D, sh], F16, name="agg16")
        u16 = big.tile([D, sh], F16, name="u16")
        v16 = big.tile([D, sh], F16, name="v16")
        xr16 = big.tile([D, sh], F16, name="xr16")
        nc.sync.dma_start(out=xr16[:], in_=x0_d[:])

        def bn_coeffs(l, j, s2, a_out, c_out):
            nc.sync.dma_start(out=bn_in[(l, j)][:], in_=s2[:])
            nc.gpsimd.collective_compute(
                "AllReduce", ALU.add, replica_groups=rg,
                ins=[bn_in[(l, j)][:]], outs=[bn_out[(l, j)][:]])
            sums = sp.tile([D, 2], F32, tag="sums", name=f"sums{l}{j}")
            nc.sync.dma_start(out=sums[:], in_=bn_out[(l, j)][:])
            g_sb = bn_sb[(f"g{j}", l)]
            b_sb = bn_sb[(f"b{j}", l)]
            m = sp.tile([D, 1], F32, tag="bnv", name="m")
            msq = sp.tile([D, 1], F32, tag="bnv", name="msq")
            var = sp.tile([D, 1], F32, tag="bnv", name="var")
            r_ = sp.tile([D, 1], F32, tag="bnv", name="r")
            nc.vector.tensor_scalar_mul(out=m[:], in0=sums[:, 0:1],
                                        scalar1=1.0 / N)
            nc.vector.tensor_scalar_mul(out=msq[:], in0=sums[:, 1:2],
                                        scalar1=1.0 / N)
            nc.vector.tensor_tensor(out=var[:], in0=m[:], in1=m[:], op=ALU.mult)
            nc.vector.tensor_tensor(out=var[:], in0=msq[:], in1=var[:],
                                    op=ALU.subtract)
            nc.vector.tensor_scalar_add(out=var[:], in0=var[:], scalar1=EPS)
            nc.vector.reciprocal(out=r_[:], in_=var[:])
            nc.scalar.activation(out=a_out[:], in_=r_[:], func=AF.Sqrt)
            nc.vector.tensor_tensor(out=a_out[:], in0=g_sb[:], in1=a_out[:],
                                    op=ALU.mult)
            nc.vector.tensor_tensor(out=c_out[:], in0=m[:], in1=a_out[:],
                                    op=ALU.mult)
            nc.vector.tensor_tensor(out=c_out[:], in0=b_sb[:], in1=c_out[:],
                                    op=ALU.subtract)

        nchunks_r = [-(-nblkR[r] // GBLK) for r in range(2)]

        for l in range(DEPTH):
            # ---- payload: stream (l=0) or batched dma_gather (l=1)
            gtiles = [[], []]
            for r in range(2):
                for j in range(nchunks_r[r]):
                    nb = min(GBLK, nblkR[r] - GBLK * j)
                    gt = gp.tile([128, GBLK * D], F16, tag=f"G{r}",
                                 name=f"G{l}_{r}_{j}")
                    gtiles[r].append(gt)
                    if l == 0:
                        nc.sync.dma_start(
                            out=gt[:, :nb * D],
                            in_=pay_d[r][:, GBLK * j * D:(GBLK * j + nb) * D])
                    else:
                        view = vtabH1[:] if r == 0 else vtabH2[:]
                        nidx = nb * BLK
                        nc.gpsimd.dma_gather(
                            gt[:, :nb * D].rearrange("p (b d) -> p b d", d=D),
                            view, idx_sb[r][:, GBLK * 8 * j:GBLK * 8 * j + nb * 8],
                            nidx, nidx, D,
                            queue_num=(r * nchunks_r[0] + j) % 4)

            # ---- per-tile segment-matmul aggregation
            for t in range(nt):
                nbt = nsub_t[t]
                s0 = suboff[t]
                St = ck.tile([128, maxsub * 128], F16, tag="S", name=f"S{l}_{t}")
                dr = drel_sb[:, s0:s0 + nbt]
                dr_b = bass.AP(dr.tensor, dr.offset, dr.ap + [[0, 128]])
                nc.vector.tensor_tensor(
                    out=St[:, :nbt * 128].rearrange("p (b j) -> p b j", j=128),
                    in0=iota_sb[:, :nbt * 128].rearrange("p (b j) -> p b j", j=128),
                    in1=dr_b, op=ALU.is_equal)
                ps = psA.tile([128, D], F32, tag="agg", name=f"agg{l}_{t}")
                si = 0
                for r in range(2):
                    b0, nb = int(blkoff[t][r]), int(nblk_t[t][r])
                    for bi in range(nb):
                        gb = b0 + bi
                        gt = gtiles[r][gb // GBLK]
                        slot = gb % GBLK
                        nc.tensor.matmul(
                            ps[:], St[:, si * 128:(si + 1) * 128],
                            gt[:, slot * D:(slot + 1) * D],
                            start=(si == 0), stop=(si == nbt - 1))
                        si += 1
                acc2 = sp.tile([128, D], F16, tag="acc2", name=f"acc2{l}_{t}")
                nc.vector.tensor_scalar_mul(out=acc2[:], in0=ps[:],
                                            scalar1=invdeg_sb[:, t:t + 1])
                pv = psT.tile([128, 128], F16, tag="tr", name=f"tr{l}_{t}")
                nc.tensor.transpose(pv[:], acc2[:], ident16[:])
                nc.scalar.activation(out=agg16[:, t * 128:(t + 1) * 128],
                                     in_=pv[:], func=AF.Copy)

            # ---- dense sweep 1: GCN linear + residual -> u; stats of u
            ssum1 = sp.tile([D, nch], F32, tag="ssum", name=f"ssum{l}1")
            ssq1 = sp.tile([D, nch], F32, tag="ssq", name=f"ssq{l}1")
            for ci, (c0, cw) in enumerate(chunks):
                sl = slice(c0, c0 + cw)
                ph = psG.tile([D, CHUNK], F32, tag="gcn", name=f"ph{l}{c0}")
                if l == 0:
                    nc.tensor.matmul(ph[:, :cw], wg_sb[0][:], agg16[:, sl],
                                     start=True, stop=True)
                else:
                    nc.tensor.matmul(ph[:, :cw], wg1p[:], agg16[:, sl],
                                     start=True, stop=False)
                    nc.tensor.matmul(ph[:, :cw], cw2_16[:], mask_sb[:, sl],
                                     start=False, stop=True)
                nc.vector.tensor_tensor(out=u16[:, sl], in0=ph[:, :cw],
                                        in1=xr16[:, sl], op=ALU.add)
                rw = max(0, min(cw, sh_real - c0))
                if rw == 0:
                    nc.vector.memset(ssum1[:, ci:ci + 1], 0.0)
                    nc.vector.memset(ssq1[:, ci:ci + 1], 0.0)
                    continue
                nc.vector.tensor_reduce(out=ssum1[:, ci:ci + 1],
                                        in_=u16[:, c0:c0 + rw],
                                        axis=mybir.AxisListType.X, op=ALU.add)
                sq = ck.tile([D, CHUNK], F16, tag="sq", name=f"sq{l}1{ci}")
                nc.scalar.activation(out=sq[:, :rw], in_=u16[:, c0:c0 + rw],
                                     func=AF.Square,
                                     accum_out=ssq1[:, ci:ci + 1])
            s2a = sp.tile([D, 2], F32, tag="s2", name=f"s2a{l}")
            nc.vector.tensor_reduce(out=s2a[:, 0:1], in_=ssum1[:],
                                    axis=mybir.AxisListType.X, op=ALU.add)
            nc.vector.tensor_reduce(out=s2a[:, 1:2], in_=ssq1[:],
                                    axis=mybir.AxisListType.X, op=ALU.add)
            a1 = sp.tile([D, 1], F32, tag="co", name=f"a1_{l}")
            c1 = sp.tile([D, 1], F32, tag="co", name=f"c1_{l}")
            bn_coeffs(l, 1, s2a, a1, c1)

            # ---- dense sweep 2: BN1 affine -> FF -> v; stats; (l=0) vshard
            ssum2 = sp.tile([D, nch], F32, tag="ssum", name=f"ssum{l}2")
            ssq2 = sp.tile([D, nch], F32, tag="ssq", name=f"ssq{l}2")
            for ci, (c0, cw) in enumerate(chunks):
                sl = slice(c0, c0 + cw)
                xp = ck.tile([D, CHUNK], F16, tag="xp", name=f"xp{l}{c0}")
                nc.vector.tensor_scalar(out=xp[:, :cw], in0=u16[:, sl],
                                        scalar1=a1[:], scalar2=c1[:],
                                        op0=ALU.mult, op1=ALU.add)
                py = psY.tile([D, CHUNK], F32, tag="ff2", name=f"py{l}{c0}")
                for h in range(H // D):
                    pr = psF.tile([D, CHUNK], F32, tag="ff1",
                                  name=f"pr{l}{c0}{h}")
                    nc.tensor.matmul(pr[:, :cw], w1_sb[l][:, h * D:(h + 1) * D],
                                     xp[:, :cw], start=True, stop=True)
                    rh = ck.tile([D, CHUNK], F16, tag="rh", name=f"rh{l}{c0}{h}")
                    nc.scalar.activation(out=rh[:, :cw], in_=pr[:, :cw],
                                         func=AF.Relu, bias=fb1_sb[l][:, h:h + 1],
                                         scale=1.0)
                    nc.tensor.matmul(py[:, :cw], w2_sb[l][h][:], rh[:, :cw],
                                     start=(h == 0), stop=(h == H // D - 1))
                nc.vector.tensor_tensor(out=v16[:, sl], in0=py[:, :cw],
                                        in1=xp[:, :cw], op=ALU.add)
                rw = max(0, min(cw, sh_real - c0))
                if l == 0 and rw < cw:
                    nc.vector.memset(v16[:, c0 + rw:c0 + cw], 0.0)
                if rw > 0:
                    nc.vector.tensor_reduce(out=ssum2[:, ci:ci + 1],
                                            in_=v16[:, c0:c0 + rw],
                                            axis=mybir.AxisListType.X,
                                            op=ALU.add)
                    sq = ck.tile([D, CHUNK], F16, tag="sq", name=f"sq{l}2{ci}")
                    nc.scalar.activation(out=sq[:, :rw], in_=v16[:, c0:c0 + rw],
                                         func=AF.Square,
                                         accum_out=ssq2[:, ci:ci + 1])
                else:
                    nc.vector.memset(ssum2[:, ci:ci + 1], 0.0)
                    nc.vector.memset(ssq2[:, ci:ci + 1], 0.0)
                if l == 0:
                    for t in range(c0 // 128, (c0 + cw) // 128):
                        pv = psT.tile([128, 128], F16, tag="tr", name=f"tv{t}")
                        nc.tensor.transpose(pv[:], v16[:, t * 128:(t + 1) * 128],
                                            ident16[:])
                        vT = sp.tile([128, D], F16, tag="vT", name=f"vT{t}")
                        nc.scalar.activation(out=vT[:], in_=pv[:], func=AF.Copy)
                        nc.sync.dma_start(out=vshard[t * 128:(t + 1) * 128, :],
                                          in_=vT[:])
                    if c0 + cw == H1:
                        nc.gpsimd.collective_compute(
                            "AllGather", ALU.bypass, replica_groups=rg,
                            ins=[vshard[0:H1, :]], outs=[vtabH1[:]])
            s2b = sp.tile([D, 2], F32, tag="s2", name=f"s2b{l}")
            nc.vector.tensor_reduce(out=s2b[:, 0:1], in_=ssum2[:],
                                    axis=mybir.AxisListType.X, op=ALU.add)
            nc.vector.tensor_reduce(out=s2b[:, 1:2], in_=ssq2[:],
                                    axis=mybir.AxisListType.X, op=ALU.add)

            if l == 0:
                nc.gpsimd.collective_compute(
                    "AllGather", ALU.bypass, replica_groups=rg,
                    ins=[vshard[H1:sh, :]], outs=[vtabH2[:]])
                a2 = sp.tile([D, 1], F32, tag="co", name="a2_0")
                c2 = sp.tile([D, 1], F32, tag="co", name="c2_0")
                bn_coeffs(l, 2, s2b, a2, c2)
                nc.vector.tensor_scalar_mul(out=wg1p[:], in0=wg_sb[1][:],
                                            scalar1=a2[:])
                c2_16 = sp.tile([D, 1], F16, tag="c216", name="c2_16")
                nc.vector.tensor_copy(out=c2_16[:], in_=c2[:])
                pcw = psG.tile([D, CHUNK], F32, tag="gcn", name="pcw2")
                nc.tensor.matmul(pcw[0:1, 0:D], c2_16[:], wg_sb[1][:],
                                 start=True, stop=True)
                nc.scalar.activation(out=cw2_16[:], in_=pcw[0:1, 0:D],
                                     func=AF.Copy)
                nc.vector.tensor_scalar(out=xr16[:], in0=v16[:],
                                        scalar1=a2[:], scalar2=c2[:],
                                        op0=ALU.mult, op1=ALU.add)
            else:
                a2p = sp.tile([D, 1], F32, tag="co", name="a2_1")
                c2p = sp.tile([D, 1], F32, tag="co", name="c2_1")
                bn_coeffs(l, 2, s2b, a2p, c2p)
                nc.vector.tensor_scalar_mul(out=clsw2[:], in0=clsw_sb[:],
                                            scalar1=a2p[:])
                c2p_16 = sp.tile([D, 1], F16, tag="c216", name="c2p_16")
                nc.vector.tensor_copy(out=c2p_16[:], in_=c2p[:])
                pcb = psY.tile([D, CHUNK], F32, tag="ff2", name="pcb")
                nc.tensor.matmul(pcb[0:16, 0:1], clsw_sb[:], c2p_16[:],
                                 start=True, stop=True)
                nc.vector.tensor_tensor(out=clsb2[:], in0=pcb[0:16, 0:1],
                                        in1=clsb_sb[:], op=ALU.add)
                for c0, cw in chunks:
                    sl = slice(c0, c0 + cw)
                    pc = psY.tile([D, CHUNK], F32, tag="ff2", name=f"pc{c0}")
                    nc.tensor.matmul(pc[0:16, :cw], clsw2[:], v16[:, sl],
                                     start=True, stop=True)
                    oc = ck.tile([16, CHUNK], F32, tag="oc", name=f"oc{c0}")
                    nc.scalar.activation(out=oc[:, :cw], in_=pc[0:16, :cw],
                                         func=AF.Identity, bias=clsb2[:],
                                         scale=1.0)
                    nc.sync.dma_start(out=out_d[:, sl], in_=oc[:, :cw])

    nc.compile()
    return nc


# ----------------------------------------------------------------------------
# Entry points
# ----------------------------------------------------------------------------

def _make_in_maps(cfg, inputs):
    W_gcn = np.asarray(inputs["W_gcn"], np.float32)
    ff_w1 = np.asarray(inputs["ff_w1"], np.float32)
    ff_b1 = np.asarray(inputs["ff_b1"], np.float32)
    ff_w2 = np.asarray(inputs["ff_w2"], np.float32)
    cls_w = np.asarray(inputs["cls_w"], np.float32)
    cls_b = np.asarray(inputs["cls_b"], np.float32)

    shared = {
        "clsw": np.ascontiguousarray(cls_w.astype(np.float16)),
        "clsb": np.ascontiguousarray(cls_b.reshape(16, 1)),
        "iotar": cfg["iota_rep"],
        "ident16": np.ascontiguousarray(np.eye(128, dtype=np.float16)),
    }
    for l in range(DEPTH):
        shared[f"wg{l}"] = np.ascontiguousarray(W_gcn[l].astype(np.float16))
        shared[f"w1_{l}"] = np.ascontiguousarray(ff_w1[l].astype(np.float16))
        shared[f"fb1_{l}"] = np.ascontiguousarray(
            ff_b1[l].reshape(H // D, D).T)
        shared[f"w2_{l}"] = np.ascontiguousarray(ff_w2[l].astype(np.float16))
        shared[f"g1_{l}"] = np.ascontiguousarray(
            np.asarray(inputs["bn1_g"], np.float32)[l].reshape(D, 1))
        shared[f"b1_{l}"] = np.ascontiguousarray(
            np.asarray(inputs["bn1_b"], np.float32)[l].reshape(D, 1))
        shared[f"g2_{l}"] = np.ascontiguousarray(
            np.asarray(inputs["bn2_g"], np.float32)[l].reshape(D, 1))
        shared[f"b2_{l}"] = np.ascontiguousarray(
            np.asarray(inputs["bn2_b"], np.float32)[l].reshape(D, 1))

    sh = cfg["sh"]
    in_maps = []
    for c in range(CORES):
        m = dict(shared)
        m["x016"] = np.ascontiguousarray(
            cfg["table0"][c * sh:(c + 1) * sh].T.astype(np.float16))
        m["pay0"] = cfg["pay"][0][c]
        m["pay1"] = cfg["pay"][1][c]
        m["idx0"] = cfg["idxw"][0][c]
        m["idx1"] = cfg["idxw"][1][c]
        m["drel"] = np.ascontiguousarray(cfg["drel"][c])
        m["invdeg"] = np.ascontiguousarray(cfg["invdeg"][c])
        m["mask16"] = np.ascontiguousarray(cfg["mask"][c].reshape(1, sh))
        in_maps.append(m)
    return in_maps


def _postprocess(cfg, results):
    sh, sh_real = cfg["sh"], cfg["sh_real"]
    N = cfg["N"]
    node_of_tok = cfg["node_of_tok"]
    out = np.empty((N, 16), np.float32)
    for c in range(CORES):
        arr = results[c]["out_fm"]
        toks = np.arange(c * sh, c * sh + sh_real)
        out[node_of_tok[toks]] = arr.T[:sh_real]
    return out


def _ensure_axon_hooks():
    try:
        import antenv.axon_hooks  # noqa: F401
        return
    except ImportError:
        pass
    import types
    import antenv
    mod = types.ModuleType("antenv.axon_hooks")
    mod._hook = None

    def set_axon_ntff_profile_hook(h):
        mod._hook = h

    def get_axon_ntff_profile_hook():
        return mod._hook

    mod.set_axon_ntff_profile_hook = set_axon_ntff_profile_hook
    mod.get_axon_ntff_profile_hook = get_axon_ntff_profile_hook
    sys.modules["antenv.axon_hooks"] = mod
    antenv.axon_hooks = mod
    try:
        from trn_agent_boot.trn_boot import _ntff_profile_via_ctypes
        h = _ntff_profile_via_ctypes("/opt/axon/libaxon_pjrt.so")
        if h is not None:
            mod._hook = h
    except Exception as e:  # pragma: no cover
        print(f"ntff hook setup failed: {e}", file=sys.stderr)


_CACHE = {}


def run(trace=False, **inputs):
    if trace:
        _ensure_axon_hooks()
    nodes = np.asarray(inputs["nodes"], np.float32)
    edge_src = np.asarray(inputs["edge_src"], np.int64)
    edge_dst = np.asarray(inputs["edge_dst"], np.int64)
    cfg = _prepare(nodes, edge_src, edge_dst)

    key = (nodes.shape, len(edge_src), int(cfg["suboff"][-1]))
    if key not in _CACHE:
        _CACHE[key] = build_program(cfg)
    nc = _CACHE[key]

    in_maps = _make_in_maps(cfg, inputs)
    res = run_bass_kernel_spmd(nc, in_maps, list(range(CORES)), trace=trace)
    return _postprocess(cfg, res.results), res


def kernel(**inputs) -> np.ndarray:
    out, _ = run(trace=False, **inputs)
    return out
